# revision 10
# baseline (speedup 1.0000x reference)
"""Trainium2 Bass kernel for nn_NeuralLongTermMemory.

Sharding: tokens (B*S = 32768) split 8 ways -> 4096 tokens/core (half a
batch-sequence each).  All weights replicated.  Gradients of the memory
MLP are partial-summed per core and combined with one ReduceScatter; the
data-dependent scalar gates use one small AllReduce of per-batch x sums.

Layout: feature-major ("transposed") everywhere - features on SBUF
partitions (grouped [128, G, cols]), tokens on the free dimension.  The
causal depthwise conv then becomes shifted-window ops on the free dim.
The gradient outer-products need token-major operands; those are
produced with PE transposes.

All matmuls run in float32r (full-rate fp32, ~1e-4 rel rounding).
"""

import numpy as np
import concourse.bass as bass
import concourse.mybir as mybir
import concourse.tile as tile
from concourse import bacc
from concourse.bass_utils import run_bass_kernel_spmd

f32 = mybir.dt.float32
f32r = mybir.dt.float32r
AF = mybir.ActivationFunctionType
ALU = mybir.AluOpType

B, S, DIM, HID, K = 4, 8192, 512, 1024, 4
N_CORES = 8
T = B * S // N_CORES        # 4096 tokens per core
HALO = K - 1                # 3
PAD = 4                     # input halo columns (even matmul free dims)
C = 256                     # chunk tokens
NCH = T // C
GD = DIM // 128             # 4
GO = HID // 128             # 8
C_LOSS = 2.0 / (B * S * DIM)
MEM_LR, MEM_MOM = 0.01, 0.9
EPS = 1e-12
SL1 = 128 * 512             # g1 slice elems per core
REPLICA = [list(range(N_CORES))]

_CACHE = {}


def _pst(ps, shape, tag, bufs, dt=f32):
    return ps.tile(shape, dt, name=tag, tag=tag, bufs=bufs, space="PSUM")


def _build():
    nc = bacc.Bacc("TRN2", target_bir_lowering=False, debug=False,
                   num_devices=N_CORES)

    def din(name, shape, dt=f32r):
        return nc.dram_tensor(name, shape, dt, kind="ExternalInput")

    def dout(name, shape):
        return nc.dram_tensor(name, shape, f32, kind="ExternalOutput")

    xT = din("xT", [128, GD, T + PAD])
    w1T = {t: din(f"w1T_{t}", [128, GD, DIM]) for t in "kvq"}
    w2T = {t: din(f"w2T_{t}", [128, GD, DIM]) for t in "kvq"}
    mw1T_d = din("mw1T", [128, GD, HID])
    mw2T_d = din("mw2T", [128, GO, DIM])
    mw2n_d = din("mw2n", [128, GD, HID])
    woutT_d = din("woutT", [128, GD, DIM])
    gwT_d = din("gwT", [128, GD, 3, DIM], f32)
    ident_d = din("ident", [128, 128])
    ones_d = din("ones", [128, 129])
    convw_d = din("convw", [128, GD, 3, K], f32)
    convb_d = din("convb", [128, GD, 3], f32)
    gateb_d = din("gateb", [128, GD, 3], f32)
    sel_d = din("sel", [128, B], f32)
    mw1s_d = din("mw1s", [128, DIM], f32)
    mom1s_d = din("mom1s", [128, DIM], f32)
    mw2s_d = din("mw2s", [64, HID], f32)
    mom2s_d = din("mom2s", [64, HID], f32)

    outT = dout("outT", [128, GD, T])
    nw1_s = dout("nw1_s", [128, DIM])
    s1_s = dout("s1_s", [128, DIM])
    nw2_s = dout("nw2_s", [64, HID])
    s2_s = dout("s2_s", [64, HID])

    # collective scratch
    cc_g_in = nc.dram_tensor("cc_g_in", [N_CORES, 2 * SL1], f32)
    cc_g_out = nc.dram_tensor("cc_g_out", [2 * SL1], f32)
    cc_x_in = nc.dram_tensor("cc_x_in", [128, GD * B], f32)
    cc_x_out = nc.dram_tensor("cc_x_out", [128, GD * B], f32,
                              addr_space="Shared")

    with tile.TileContext(nc) as tc:
        _emit(nc, tc, locals())
    nc.compile()
    return nc


def _emit(nc, tc, d):
    sc, ve, te, sy, gp = nc.scalar, nc.vector, nc.tensor, nc.sync, nc.gpsimd

    with (
        tc.tile_pool(name="consts", bufs=1) as consts,
        tc.tile_pool(name="accs", bufs=1) as accs,
        tc.tile_pool(name="wmem", bufs=1) as wmem,
        tc.tile_pool(name="ps", bufs=1, space="PSUM") as ps,
    ):
        # ---- constants / resident weights ----
        ident = consts.tile([128, 128], f32r)
        sy.dma_start(ident[:], d["ident_d"][:])
        onesb = consts.tile([128, 129], f32r)
        sy.dma_start(onesb[:], d["ones_d"][:])
        ones_col = onesb[:, 0:1]
        ones_row = onesb[0:1, 1:129]
        ones_col32 = consts.tile([128, 1], f32)
        gp.memset(ones_col32[:], 1.0)
        convw = consts.tile([128, GD, 3, K], f32)
        sy.dma_start(convw[:], d["convw_d"][:])
        convb = consts.tile([128, GD, 3], f32)
        sy.dma_start(convb[:], d["convb_d"][:])
        sel = consts.tile([128, B], f32)
        sy.dma_start(sel[:], d["sel_d"][:])

        mw1T = wmem.tile([128, GD, HID], f32r)
        sy.dma_start(mw1T[:], d["mw1T_d"][:])
        mw2T = wmem.tile([128, GO, DIM], f32r)
        sy.dma_start(mw2T[:], d["mw2T_d"][:])
        mw2n = wmem.tile([128, GD, HID], f32r)
        sy.dma_start(mw2n[:], d["mw2n_d"][:])

        g1acc = accs.tile([128, GO, DIM], f32)
        gp.memset(g1acc[:], 0.0)
        g2acc = accs.tile([128, GD, HID], f32)
        gp.memset(g2acc[:], 0.0)
        xsum = accs.tile([128, GD, 1], f32)
        gp.memset(xsum[:], 0.0)

        w1sb = {}
        w2sb = {}

        def load_proj_weights(pool, tensors):
            for t in tensors:
                wa = pool.tile([128, GD, DIM], f32r, name=f"w1sb_{t}",
                               tag=f"w1sb_{t}")
                sy.dma_start(wa[:], d["w1T"][t][:])
                w1sb[t] = wa
                wb = pool.tile([128, GD, DIM], f32r, name=f"w2sb_{t}",
                               tag=f"w2sb_{t}")
                sy.dma_start(wb[:], d["w2T"][t][:])
                w2sb[t] = wb

        def proj_conv_silu(tn, xc, out_t):
            ncols = C + PAD
            y1s = work.tile([128, GD, ncols], f32r, name="y1s", tag="y1s",
                            bufs=2)
            for gj in range(GD):
                p = _pst(ps, [128, ncols], "mm", 2)
                for gd in range(GD):
                    te.matmul(p[:], w1sb[tn][:, gd, gj * 128:(gj + 1) * 128],
                              xc[:, gd, :], start=(gd == 0), stop=(gd == GD - 1))
                sc.activation(y1s[:, gj, :], p[:], AF.Silu)
            ca = work.tile([128, GD, C], f32, name="ca", tag="ca", bufs=2)
            ti = "kvq".index(tn)
            for gj in range(GD):
                p = _pst(ps, [128, ncols], "mm", 2)
                for gd in range(GD):
                    te.matmul(p[:], w2sb[tn][:, gd, gj * 128:(gj + 1) * 128],
                              y1s[:, gd, :], start=(gd == 0), stop=(gd == GD - 1))
                ve.tensor_scalar(ca[:, gj, :], p[:, 1:1 + C],
                                 convw[:, gj, ti, 0:1], None, ALU.mult)
                for kk in range(1, K):
                    ve.scalar_tensor_tensor(ca[:, gj, :], p[:, 1 + kk:1 + kk + C],
                                            convw[:, gj, ti, kk:kk + 1],
                                            ca[:, gj, :], ALU.mult, ALU.add)
            for gj in range(GD):
                sc.activation(out_t[:, gj, :], ca[:, gj, :], AF.Silu,
                              bias=convb[:, gj, ti:ti + 1])

        def l2norm_inplace(src):
            """src: [128, GD, C] fp32r silu output; normalized in place."""
            sq = work.tile([128, GD, C], f32r, name="sq", tag="sq")
            gp.tensor_mul(sq[:], src[:], src[:])
            ssp = _pst(ps, [1, C], "mm", 2)
            for gd in range(GD):
                te.matmul(ssp[:], ones_col, sq[:, gd, :],
                          start=(gd == 0), stop=(gd == GD - 1))
            with nc.allow_low_precision("f32r norm factor"):
                rnr = work.tile([1, C], f32r, name="rnr", tag="rnr")
                sc.activation(rnr[:], ssp[:], AF.Sqrt)
                ve.tensor_scalar_max(rnr[:], rnr[:], EPS)
                bc = _pst(ps, [128, C], "mm", 2)
                te.matmul(bc[:], ones_row, rnr[:])
                rnb = work.tile([128, C], f32r, name="rnb", tag="rnb")
                ve.reciprocal(rnb[:], bc[:])
            for gd in range(GD):
                ve.tensor_mul(src[:, gd, :], src[:, gd, :], rnb[:])

        # ============ PASS 1: k/v + gradient accumulation ============
        work_cm = tc.tile_pool(name="work", bufs=1)
        work = work_cm.__enter__()
        with tc.tile_pool(name="w_kv", bufs=1) as w_kv:
            load_proj_weights(w_kv, "kv")
            for ci in range(NCH):
                xc = work.tile([128, GD, C + PAD], f32r, name="xc", tag="xc",
                               bufs=2)
                sy.dma_start(xc[:], d["xT"][:, :, ci * C:ci * C + C + PAD])
                # gate statistics: sum x over this chunk's tokens
                xs_c = work.tile([128, GD, 1], f32, name="xs_c", tag="xs_c")
                ve.tensor_reduce(xs_c[:], xc[:, :, PAD:], mybir.AxisListType.X,
                                 ALU.add)
                ve.tensor_add(xsum[:], xsum[:], xs_c[:])

                k_n = work.tile([128, GD, C], f32r, name="k_n", tag="k_n")
                proj_conv_silu("k", xc, k_n)
                vs = work.tile([128, GD, C], f32r, name="vs", tag="vs")
                proj_conv_silu("v", xc, vs)
                l2norm_inplace(k_n)

                # forward through memory MLP (feature-major)
                a1 = work.tile([128, GO, C], f32r, name="a1", tag="a1")
                sp1 = work.tile([128, GO, C], f32, name="sp1", tag="sp1")
                ph = _pst(ps, [128, GO, C], "h4", 1)
                for go in range(GO):
                    for gd in range(GD):
                        te.matmul(ph[:, go, :],
                                  mw1T[:, gd, go * 128:(go + 1) * 128],
                                  k_n[:, gd, :], start=(gd == 0),
                                  stop=(gd == GD - 1))
                sc.activation(a1[:], ph[:], AF.Silu)
                sc.activation(sp1[:], ph[:], AF.Derivative_silu)
                dpredu = work.tile([128, GD, C], f32r, name="dpredu",
                                   tag="dpredu")
                pp = _pst(ps, [128, GD, C], "h4", 1)
                for gi in range(GD):
                    for go in range(GO):
                        te.matmul(pp[:, gi, :],
                                  mw2T[:, go, gi * 128:(gi + 1) * 128],
                                  a1[:, go, :], start=(go == 0),
                                  stop=(go == GO - 1))
                ve.tensor_sub(dpredu[:], pp[:], vs[:])
                dh1 = work.tile([128, GO, C], f32r, name="dh1", tag="dh1")
                pd = _pst(ps, [128, GO, C], "h4", 1)
                for go in range(GO):
                    for gi in range(GD):
                        te.matmul(pd[:, go, :],
                                  mw2n[:, gi, go * 128:(go + 1) * 128],
                                  dpredu[:, gi, :], start=(gi == 0),
                                  stop=(gi == GD - 1))
                ve.tensor_mul(dh1[:], pd[:], sp1[:])

                # transpose to token-major for the outer products
                nsub = C // 128
                k_tm = work.tile([128, nsub, DIM], f32r, name="k_tm",
                                 tag="k_tm")
                dp_tm = work.tile([128, nsub, DIM], f32r, name="dp_tm",
                                  tag="dp_tm")
                a1_tm = work.tile([128, nsub, HID], f32r, name="a1_tm",
                                  tag="a1_tm")
                dh1_tm = work.tile([128, nsub, HID], f32r, name="dh1_tm",
                                   tag="dh1_tm")
                for sub in range(nsub):
                    for src, dst, ng in ((k_n, k_tm, GD), (dpredu, dp_tm, GD),
                                         (a1, a1_tm, GO), (dh1, dh1_tm, GO)):
                        for h in range(ng // 4):
                            tp = _pst(ps, [128, 512], "sm", 2, f32r)
                            for j in range(4):
                                g = h * 4 + j
                                te.transpose(tp[:, j * 128:(j + 1) * 128],
                                             src[:, g, sub * 128:(sub + 1) * 128],
                                             ident[:])
                            sc.activation(dst[:, sub, h * 512:(h + 1) * 512],
                                          tp[:], AF.Copy)
                # g1[o, d] += dh1_tm.T @ k_tm ; g2[i, o] += dp_tm.T @ a1_tm
                for go in range(GO):
                    p = _pst(ps, [128, DIM], "sm", 2)
                    for sub in range(nsub):
                        te.matmul(p[:], dh1_tm[:, sub, go * 128:(go + 1) * 128],
                                  k_tm[:, sub, :], start=(sub == 0),
                                  stop=(sub == nsub - 1))
                    ve.scalar_tensor_tensor(g1acc[:, go, :], p[:], C_LOSS,
                                            g1acc[:, go, :], ALU.mult, ALU.add)
                for gi in range(GD):
                    for h in range(2):
                        p = _pst(ps, [128, 512], "sm", 2)
                        for sub in range(nsub):
                            te.matmul(p[:],
                                      dp_tm[:, sub, gi * 128:(gi + 1) * 128],
                                      a1_tm[:, sub, h * 512:(h + 1) * 512],
                                      start=(sub == 0), stop=(sub == nsub - 1))
                        ve.scalar_tensor_tensor(g2acc[:, gi,
                                                      h * 512:(h + 1) * 512],
                                                p[:], C_LOSS,
                                                g2acc[:, gi,
                                                      h * 512:(h + 1) * 512],
                                                ALU.mult, ALU.add)

        # ---- launch collectives (overlap with pass 2) ----
        part = work.tile([128, GD, B], f32, name="part", tag="part")
        for bcol in range(B):
            ve.tensor_scalar_mul(part[:, :, bcol:bcol + 1], xsum[:],
                                 sel[:, bcol:bcol + 1])
        sy.dma_start(d["cc_x_in"][:].rearrange("p (g b) -> p g b", g=GD),
                     part[:])
        gp.collective_compute("AllReduce", ALU.add, replica_groups=REPLICA,
                              ins=[d["cc_x_in"][:]], outs=[d["cc_x_out"][:]])
        g1v = d["cc_g_in"][:, 0:SL1].rearrange("s (p d) -> p s d", p=128)
        sy.dma_start(g1v, g1acc[:])
        g2v = d["cc_g_in"][:, SL1:].rearrange(
            "(gi ph) (q o) -> ph q gi o", gi=GD, q=64)
        for ph in range(2):
            sy.dma_start(g2v[ph], g2acc[ph * 64:(ph + 1) * 64, :, :])
        gp.collective_compute("ReduceScatter", ALU.add, replica_groups=REPLICA,
                              ins=[d["cc_g_in"][:]], outs=[d["cc_g_out"][:]])
        # (work pool stays open through pass 2, closed before the tail)

        # ============ PASS 2: queries -> retrieved -> output ============
        with tc.tile_pool(name="w_q", bufs=1) as w_q:
            load_proj_weights(w_q, "q")
            woutT = w_q.tile([128, GD, DIM], f32r, name="woutT")
            sy.dma_start(woutT[:], d["woutT_d"][:])
            for ci in range(NCH):
                xc = work.tile([128, GD, C + PAD], f32r, name="xc", tag="xc",
                               bufs=2)
                sy.dma_start(xc[:], d["xT"][:, :, ci * C:ci * C + C + PAD])
                q_n = work.tile([128, GD, C], f32r, name="q_n", tag="k_n")
                proj_conv_silu("q", xc, q_n)
                l2norm_inplace(q_n)
                aq = work.tile([128, GO, C], f32r, name="aq", tag="a1")
                ph = _pst(ps, [128, GO, C], "h4", 1)
                for go in range(GO):
                    for gd in range(GD):
                        te.matmul(ph[:, go, :],
                                  mw1T[:, gd, go * 128:(go + 1) * 128],
                                  q_n[:, gd, :], start=(gd == 0),
                                  stop=(gd == GD - 1))
                sc.activation(aq[:], ph[:], AF.Silu)
                r_s = work.tile([128, GD, C], f32r, name="r_s", tag="dpredu")
                pr = _pst(ps, [128, GD, C], "h4", 1)
                for gi in range(GD):
                    for go in range(GO):
                        te.matmul(pr[:, gi, :],
                                  mw2T[:, go, gi * 128:(gi + 1) * 128],
                                  aq[:, go, :], start=(go == 0),
                                  stop=(go == GO - 1))
                sc.activation(r_s[:], pr[:], AF.Copy)
                outs = work.tile([128, GD, C], f32, name="outs", tag="vs")
                po = _pst(ps, [128, GD, C], "h4", 1)
                for gu in range(GD):
                    for gi in range(GD):
                        te.matmul(po[:, gu, :],
                                  woutT[:, gi, gu * 128:(gu + 1) * 128],
                                  r_s[:, gi, :], start=(gi == 0),
                                  stop=(gi == GD - 1))
                sc.activation(outs[:], po[:], AF.Copy)
                sy.dma_start(d["outT"][:, :, ci * C:(ci + 1) * C], outs[:])

        work_cm.__exit__(None, None, None)

        # ============ tail: gates, momentum/decay update ============
        with tc.tile_pool(name="tail", bufs=1) as tail:
            gwT = tail.tile([128, GD, 3, DIM], f32, name="gwT")
            sy.dma_start(gwT[:], d["gwT_d"][:])
            gateb = tail.tile([128, GD, 3], f32, name="gateb")
            sy.dma_start(gateb[:], d["gateb_d"][:])
            xsb = tail.tile([128, GD, B], f32, name="xsb")
            sy.dma_start(xsb[:],
                         d["cc_x_out"][:].rearrange("p (g b) -> p g b", g=GD))
            gsc = tail.tile([1, 3], f32, name="gsc")
            for w in range(3):
                sig = tail.tile([128, GD, B], f32, name=f"sig{w}",
                                tag="sig")
                for gj in range(GD):
                    p = _pst(ps, [128, B], "mm", 2)
                    for gd in range(GD):
                        te.matmul(p[:],
                                  gwT[:, gd, w, gj * 128:(gj + 1) * 128],
                                  xsb[:, gd, :], start=(gd == 0),
                                  stop=(gd == GD - 1))
                    sc.activation(sig[:, gj, :], p[:], AF.Sigmoid,
                                  bias=gateb[:, gj, w:w + 1], scale=1.0 / S)
                srow = tail.tile([128, 1], f32, name=f"srow{w}", tag="srow")
                ve.tensor_reduce(srow[:], sig[:], mybir.AxisListType.XY,
                                 ALU.add)
                p = _pst(ps, [1, 1], "mm", 2)
                te.matmul(p[:], ones_col32[:], srow[:])
                sc.activation(gsc[0:1, w:w + 1], p[:], AF.Copy,
                              scale=1.0 / (DIM * B))
            ones_row32 = tail.tile([1, 128], f32, name="ones_row32")
            gp.memset(ones_row32[:], 1.0)
            gbp = _pst(ps, [128, 3], "mm", 2)
            te.matmul(gbp[:], ones_row32[:], gsc[:])
            gb = tail.tile([128, 3], f32, name="gb")
            sc.activation(gb[:], gbp[:], AF.Copy)
            oneminus = tail.tile([128, 1], f32, name="oneminus")
            ve.tensor_scalar(oneminus[:], gb[:, 0:1], -1.0, 1.0, ALU.mult,
                             ALU.add)
            theta = tail.tile([128, 1], f32, name="theta")
            ve.tensor_scalar_mul(theta[:], gb[:, 1:2], MEM_LR)
            eta = tail.tile([128, 1], f32, name="eta")
            ve.tensor_scalar_mul(eta[:], gb[:, 2:3], MEM_MOM)

            g1s = tail.tile([128, DIM], f32, name="g1s")
            sy.dma_start(g1s[:],
                         d["cc_g_out"][0:SL1].rearrange("(p d) -> p d", p=128))
            g2s = tail.tile([64, HID], f32, name="g2s")
            sy.dma_start(g2s[:],
                         d["cc_g_out"][SL1:].rearrange("(q o) -> q o", q=64))
            mw1s = tail.tile([128, DIM], f32, name="mw1s")
            sy.dma_start(mw1s[:], d["mw1s_d"][:])
            mom1s = tail.tile([128, DIM], f32, name="mom1s")
            sy.dma_start(mom1s[:], d["mom1s_d"][:])
            mw2s = tail.tile([64, HID], f32, name="mw2s")
            sy.dma_start(mw2s[:], d["mw2s_d"][:])
            mom2s = tail.tile([64, HID], f32, name="mom2s")
            sy.dma_start(mom2s[:], d["mom2s_d"][:])

            for gsl, mws, moms, s_out, nw_out, np_ in (
                    (g1s, mw1s, mom1s, d["s1_s"], d["nw1_s"], 128),
                    (g2s, mw2s, mom2s, d["s2_s"], d["nw2_s"], 64)):
                t1 = tail.tile(list(gsl.shape), f32, name="t1", tag="t1")
                ve.tensor_scalar_mul(t1[:], gsl[:], theta[0:np_, :])
                st = tail.tile(list(gsl.shape), f32, name="st", tag="st")
                ve.scalar_tensor_tensor(st[:], moms[:], eta[0:np_, :], t1[:],
                                        ALU.mult, ALU.subtract)
                nwt = tail.tile(list(gsl.shape), f32, name="nwt", tag="nwt")
                ve.scalar_tensor_tensor(nwt[:], mws[:], oneminus[0:np_, :],
                                        st[:], ALU.mult, ALU.add)
                sy.dma_start(s_out[:], st[:])
                sy.dma_start(nw_out[:], nwt[:])


# ======================= host side =======================

def _gT(w):
    """(out, in=512) torch-Linear weight -> grouped lhsT [128, 4, out]."""
    return np.ascontiguousarray(
        w.T.reshape(GD, 128, -1).transpose(1, 0, 2)).astype(np.float32)


def _g_natural(w, groups):
    """(rows, cols) -> grouped [128, groups, cols] (rows on partitions)."""
    return np.ascontiguousarray(
        w.reshape(groups, 128, -1).transpose(1, 0, 2)).astype(np.float32)


def _prep(inputs):
    x = np.asarray(inputs["x"], np.float32)
    shared = {}
    for t, (a, b_) in {"k": ("w_k1", "w_k2"), "v": ("w_v1", "w_v2"),
                       "q": ("w_q1", "w_q2")}.items():
        shared[f"w1T_{t}"] = _gT(np.asarray(inputs[a], np.float32))
        shared[f"w2T_{t}"] = _gT(np.asarray(inputs[b_], np.float32))
    mem_w1 = np.asarray(inputs["mem_w1"], np.float32)
    mem_w2 = np.asarray(inputs["mem_w2"], np.float32)
    shared["mw1T"] = _gT(mem_w1)                       # [128,4,1024]
    shared["mw2T"] = _g_natural(mem_w2.T, GO)          # [128,8,512]
    shared["mw2n"] = _g_natural(mem_w2, GD)            # [128,4,1024]
    shared["woutT"] = _gT(np.asarray(inputs["w_out"], np.float32))
    gw = np.stack([_gT(np.asarray(inputs[f"gate_{t}_w"], np.float32))
                   for t in ("d", "lr", "m")], axis=2)  # [128,4,3,512]
    shared["gwT"] = np.ascontiguousarray(gw)
    shared["ident"] = np.eye(128, dtype=np.float32)
    shared["ones"] = np.ones((128, 129), np.float32)
    cw = np.stack([np.asarray(inputs[f"conv_{t}_w"], np.float32)[:, 0, :]
                   for t in ("k", "v", "q")], axis=1)   # (512, 3, 4)
    shared["convw"] = np.ascontiguousarray(
        cw.reshape(GD, 128, 3, K).transpose(1, 0, 2, 3))
    cb = np.stack([np.asarray(inputs[f"conv_{t}_b"], np.float32)
                   for t in ("k", "v", "q")], axis=1)   # (512, 3)
    shared["convb"] = np.ascontiguousarray(
        cb.reshape(GD, 128, 3).transpose(1, 0, 2))
    gb = np.stack([np.asarray(inputs[f"gate_{t}_b"], np.float32)
                   for t in ("d", "lr", "m")], axis=1)  # (512, 3)
    shared["gateb"] = np.ascontiguousarray(
        gb.reshape(GD, 128, 3).transpose(1, 0, 2))
    mom1 = np.asarray(inputs["mom1"], np.float32)
    mom2 = np.asarray(inputs["mom2"], np.float32)

    in_maps = []
    for c in range(N_CORES):
        b = c // 2
        half = c % 2
        seq = x[b]
        if half == 0:
            seg = np.concatenate(
                [np.zeros((PAD, DIM), np.float32), seq[0:T]], axis=0)
        else:
            seg = seq[T - PAD:2 * T]
        xTg = np.ascontiguousarray(
            seg.T.reshape(GD, 128, T + PAD).transpose(1, 0, 2))
        sel = np.zeros((128, B), np.float32)
        sel[:, b] = 1.0
        m = dict(shared)
        m["xT"] = xTg
        m["sel"] = sel
        m["mw1s"] = np.ascontiguousarray(mem_w1[c * 128:(c + 1) * 128])
        m["mom1s"] = np.ascontiguousarray(mom1[c * 128:(c + 1) * 128])
        m["mw2s"] = np.ascontiguousarray(mem_w2[c * 64:(c + 1) * 64])
        m["mom2s"] = np.ascontiguousarray(mom2[c * 64:(c + 1) * 64])
        in_maps.append(m)
    return in_maps


def _unshard(results):
    outs = []
    for c in range(N_CORES):
        a = results[c]["outT"]          # [128, 4, 4096]
        outs.append(np.moveaxis(a, 1, 0).reshape(DIM, T).T)
    output = np.concatenate(outs, axis=0).reshape(B, S, DIM)
    nw1 = np.concatenate([results[c]["nw1_s"] for c in range(N_CORES)], axis=0)
    s1 = np.concatenate([results[c]["s1_s"] for c in range(N_CORES)], axis=0)
    nw2 = np.concatenate([results[c]["nw2_s"] for c in range(N_CORES)], axis=0)
    s2 = np.concatenate([results[c]["s2_s"] for c in range(N_CORES)], axis=0)
    return (output, nw1, nw2, s1, s2)


def run(in_maps, trace=False):
    if "nc" not in _CACHE:
        _CACHE["nc"] = _build()
    return run_bass_kernel_spmd(_CACHE["nc"], in_maps,
                                core_ids=list(range(N_CORES)), trace=trace)


def kernel(**inputs):
    res = run(_prep(inputs))
    return _unshard(res.results)


if __name__ == "__main__":
    import jax
    import reference as R
    inp = {k: np.asarray(v) for k, v in R.setup_inputs().items()}
    got = kernel(**inp)
    print([g.shape for g in got])


# revision 11
# speedup vs baseline: 1.0051x; 1.0051x over previous
"""Trainium2 Bass kernel for nn_NeuralLongTermMemory.

Sharding: tokens (B*S = 32768) split 8 ways -> 4096 tokens/core (half a
batch-sequence each).  All weights replicated.  Gradients of the memory
MLP are partial-summed per core and combined with one ReduceScatter; the
data-dependent scalar gates use one small AllReduce of per-batch x sums.

Layout: feature-major ("transposed") everywhere - features on SBUF
partitions (grouped [128, G, cols]), tokens on the free dimension.  The
causal depthwise conv then becomes shifted-window ops on the free dim.
The gradient outer-products need token-major operands; those are
produced with PE transposes.

All matmuls run in float32r (full-rate fp32, ~1e-4 rel rounding).
"""

import numpy as np
import concourse.bass as bass
import concourse.mybir as mybir
import concourse.tile as tile
from concourse import bacc
from concourse.bass_utils import run_bass_kernel_spmd

f32 = mybir.dt.float32
f32r = mybir.dt.float32r
f16 = mybir.dt.float16
AF = mybir.ActivationFunctionType
ALU = mybir.AluOpType

B, S, DIM, HID, K = 4, 8192, 512, 1024, 4
N_CORES = 8
T = B * S // N_CORES        # 4096 tokens per core
HALO = K - 1                # 3
PAD = 4                     # input halo columns (even matmul free dims)
C = 256                     # chunk tokens
NCH = T // C
GD = DIM // 128             # 4
GO = HID // 128             # 8
C_LOSS = 2.0 / (B * S * DIM)
MEM_LR, MEM_MOM = 0.01, 0.9
EPS = 1e-12
SL1 = 128 * 512             # g1 slice elems per core
REPLICA = [list(range(N_CORES))]

_CACHE = {}


def _pst(ps, shape, tag, bufs, dt=f32):
    return ps.tile(shape, dt, name=tag, tag=tag, bufs=bufs, space="PSUM")


def _build():
    nc = bacc.Bacc("TRN2", target_bir_lowering=False, debug=False,
                   num_devices=N_CORES)

    def din(name, shape, dt=f16):
        return nc.dram_tensor(name, shape, dt, kind="ExternalInput")

    def dout(name, shape):
        return nc.dram_tensor(name, shape, f32, kind="ExternalOutput")

    xT = din("xT", [128, GD, T + PAD])
    w1T = {t: din(f"w1T_{t}", [128, GD, DIM]) for t in "kvq"}
    w2T = {t: din(f"w2T_{t}", [128, GD, DIM]) for t in "kvq"}
    mw1T_d = din("mw1T", [128, GD, HID])
    mw2T_d = din("mw2T", [128, GO, DIM])
    mw2n_d = din("mw2n", [128, GD, HID])
    woutT_d = din("woutT", [128, GD, DIM])
    gwT_d = din("gwT", [128, GD, 3, DIM], f32)
    ident_d = din("ident", [128, 128])
    ones_d = din("ones", [128, 129])
    convw_d = din("convw", [128, GD, 3, K], f32)
    convb_d = din("convb", [128, GD, 3], f32)
    gateb_d = din("gateb", [128, GD, 3], f32)
    sel_d = din("sel", [128, B], f32)
    mw1s_d = din("mw1s", [128, DIM], f32)
    mom1s_d = din("mom1s", [128, DIM], f32)
    mw2s_d = din("mw2s", [64, HID], f32)
    mom2s_d = din("mom2s", [64, HID], f32)

    outT = dout("outT", [128, GD, T])
    nw1_s = dout("nw1_s", [128, DIM])
    s1_s = dout("s1_s", [128, DIM])
    nw2_s = dout("nw2_s", [64, HID])
    s2_s = dout("s2_s", [64, HID])

    # collective scratch
    cc_g_in = nc.dram_tensor("cc_g_in", [N_CORES, 2 * SL1], f32)
    cc_g_out = nc.dram_tensor("cc_g_out", [2 * SL1], f32)
    cc_x_in = nc.dram_tensor("cc_x_in", [128, GD * B], f32)
    cc_x_out = nc.dram_tensor("cc_x_out", [128, GD * B], f32,
                              addr_space="Shared")

    with tile.TileContext(nc) as tc:
        _emit(nc, tc, locals())
    nc.compile()
    return nc


def _emit(nc, tc, d):
    sc, ve, te, sy, gp = nc.scalar, nc.vector, nc.tensor, nc.sync, nc.gpsimd

    with (
        tc.tile_pool(name="consts", bufs=1) as consts,
        tc.tile_pool(name="accs", bufs=1) as accs,
        tc.tile_pool(name="wmem", bufs=1) as wmem,
        tc.tile_pool(name="ps", bufs=1, space="PSUM") as ps,
    ):
        # ---- constants / resident weights ----
        ident = consts.tile([128, 128], f16)
        sy.dma_start(ident[:], d["ident_d"][:])
        onesb = consts.tile([128, 129], f16)
        sy.dma_start(onesb[:], d["ones_d"][:])
        ones_col = onesb[:, 0:1]
        ones_row = onesb[0:1, 1:129]
        ones_col32 = consts.tile([128, 1], f32)
        gp.memset(ones_col32[:], 1.0)
        convw = consts.tile([128, GD, 3, K], f32)
        sy.dma_start(convw[:], d["convw_d"][:])
        convb = consts.tile([128, GD, 3], f32)
        sy.dma_start(convb[:], d["convb_d"][:])
        sel = consts.tile([128, B], f32)
        sy.dma_start(sel[:], d["sel_d"][:])

        mw1T = wmem.tile([128, GD, HID], f16)
        sy.dma_start(mw1T[:], d["mw1T_d"][:])
        mw2T = wmem.tile([128, GO, DIM], f16)
        sy.dma_start(mw2T[:], d["mw2T_d"][:])
        mw2n = wmem.tile([128, GD, HID], f16)
        sy.dma_start(mw2n[:], d["mw2n_d"][:])

        g1acc = accs.tile([128, GO, DIM], f32)
        gp.memset(g1acc[:], 0.0)
        g2acc = accs.tile([128, GD, HID], f32)
        gp.memset(g2acc[:], 0.0)
        xsum = accs.tile([128, GD, 1], f32)
        gp.memset(xsum[:], 0.0)

        w1sb = {}
        w2sb = {}

        def load_proj_weights(pool, tensors):
            for t in tensors:
                wa = pool.tile([128, GD, DIM], f16, name=f"w1sb_{t}",
                               tag=f"w1sb_{t}")
                sy.dma_start(wa[:], d["w1T"][t][:])
                w1sb[t] = wa
                wb = pool.tile([128, GD, DIM], f16, name=f"w2sb_{t}",
                               tag=f"w2sb_{t}")
                sy.dma_start(wb[:], d["w2T"][t][:])
                w2sb[t] = wb

        def proj_conv_silu(tn, xc, out_t):
            ncols = C + PAD
            y1s = work.tile([128, GD, ncols], f16, name="y1s", tag="y1s",
                            bufs=2)
            for gj in range(GD):
                p = _pst(ps, [128, ncols], "mm", 2)
                for gd in range(GD):
                    te.matmul(p[:], w1sb[tn][:, gd, gj * 128:(gj + 1) * 128],
                              xc[:, gd, :], start=(gd == 0), stop=(gd == GD - 1))
                sc.activation(y1s[:, gj, :], p[:], AF.Silu)
            ca = work.tile([128, GD, C], f32, name="ca", tag="ca", bufs=2)
            ti = "kvq".index(tn)
            for gj in range(GD):
                p = _pst(ps, [128, ncols], "mm", 2)
                for gd in range(GD):
                    te.matmul(p[:], w2sb[tn][:, gd, gj * 128:(gj + 1) * 128],
                              y1s[:, gd, :], start=(gd == 0), stop=(gd == GD - 1))
                ve.tensor_scalar(ca[:, gj, :], p[:, 1:1 + C],
                                 convw[:, gj, ti, 0:1], None, ALU.mult)
                for kk in range(1, K):
                    ve.scalar_tensor_tensor(ca[:, gj, :], p[:, 1 + kk:1 + kk + C],
                                            convw[:, gj, ti, kk:kk + 1],
                                            ca[:, gj, :], ALU.mult, ALU.add)
            for gj in range(GD):
                sc.activation(out_t[:, gj, :], ca[:, gj, :], AF.Silu,
                              bias=convb[:, gj, ti:ti + 1])

        def l2norm_inplace(src):
            """src: [128, GD, C] fp32r silu output; normalized in place."""
            sq = work.tile([128, GD, C], f16, name="sq", tag="sq")
            gp.tensor_mul(sq[:], src[:], src[:])
            ssp = _pst(ps, [1, C], "mm", 2)
            for gd in range(GD):
                te.matmul(ssp[:], ones_col, sq[:, gd, :],
                          start=(gd == 0), stop=(gd == GD - 1))
            rn = work.tile([1, C], f32, name="rn", tag="rn")
            sc.activation(rn[:], ssp[:], AF.Sqrt)
            ve.tensor_scalar_max(rn[:], rn[:], EPS)
            with nc.allow_low_precision("norm factor to f16"):
                rnr = work.tile([1, C], f16, name="rnr", tag="rnr")
                ve.reciprocal(rnr[:], rn[:])
            rnb = work.tile([128, C], f16, name="rnb", tag="rnb")
            gp.partition_broadcast(rnb[:], rnr[:])
            for gd in range(GD):
                ve.tensor_mul(src[:, gd, :], src[:, gd, :], rnb[:])

        # ============ PASS 1: k/v + gradient accumulation ============
        work_cm = tc.tile_pool(name="work", bufs=1)
        work = work_cm.__enter__()
        with tc.tile_pool(name="w_kv", bufs=1) as w_kv:
            load_proj_weights(w_kv, "kv")
            for ci in range(NCH):
                xc = work.tile([128, GD, C + PAD], f16, name="xc", tag="xc",
                               bufs=2)
                sy.dma_start(xc[:], d["xT"][:, :, ci * C:ci * C + C + PAD])
                # gate statistics: sum x over this chunk's tokens
                xs_c = work.tile([128, GD, 1], f32, name="xs_c", tag="xs_c")
                ve.tensor_reduce(xs_c[:], xc[:, :, PAD:], mybir.AxisListType.X,
                                 ALU.add)
                ve.tensor_add(xsum[:], xsum[:], xs_c[:])

                k_n = work.tile([128, GD, C], f16, name="k_n", tag="k_n")
                proj_conv_silu("k", xc, k_n)
                vs = work.tile([128, GD, C], f16, name="vs", tag="vs")
                proj_conv_silu("v", xc, vs)
                l2norm_inplace(k_n)

                # forward through memory MLP (feature-major)
                a1 = work.tile([128, GO, C], f16, name="a1", tag="a1")
                sp1 = work.tile([128, GO, C], f16, name="sp1", tag="sp1")
                ph = _pst(ps, [128, GO, C], "h4", 1)
                for go in range(GO):
                    for gd in range(GD):
                        te.matmul(ph[:, go, :],
                                  mw1T[:, gd, go * 128:(go + 1) * 128],
                                  k_n[:, gd, :], start=(gd == 0),
                                  stop=(gd == GD - 1))
                sc.activation(a1[:], ph[:], AF.Silu)
                sc.activation(sp1[:], ph[:], AF.Derivative_silu)
                dpredu = work.tile([128, GD, C], f16, name="dpredu",
                                   tag="dpredu")
                pp = _pst(ps, [128, GD, C], "h4", 1)
                for gi in range(GD):
                    for go in range(GO):
                        te.matmul(pp[:, gi, :],
                                  mw2T[:, go, gi * 128:(gi + 1) * 128],
                                  a1[:, go, :], start=(go == 0),
                                  stop=(go == GO - 1))
                ve.tensor_sub(dpredu[:], pp[:], vs[:])
                dh1 = work.tile([128, GO, C], f16, name="dh1", tag="dh1")
                pd = _pst(ps, [128, GO, C], "h4", 1)
                for go in range(GO):
                    for gi in range(GD):
                        te.matmul(pd[:, go, :],
                                  mw2n[:, gi, go * 128:(go + 1) * 128],
                                  dpredu[:, gi, :], start=(gi == 0),
                                  stop=(gi == GD - 1))
                ve.tensor_mul(dh1[:], pd[:], sp1[:])

                # transpose to token-major for the outer products
                nsub = C // 128
                k_tm = work.tile([128, nsub, DIM], f16, name="k_tm",
                                 tag="k_tm")
                dp_tm = work.tile([128, nsub, DIM], f16, name="dp_tm",
                                  tag="dp_tm")
                a1_tm = work.tile([128, nsub, HID], f16, name="a1_tm",
                                  tag="a1_tm")
                dh1_tm = work.tile([128, nsub, HID], f16, name="dh1_tm",
                                   tag="dh1_tm")
                for sub in range(nsub):
                    for src, dst, ng in ((k_n, k_tm, GD), (dpredu, dp_tm, GD),
                                         (a1, a1_tm, GO), (dh1, dh1_tm, GO)):
                        for h in range(ng // 4):
                            tp = _pst(ps, [128, 512], "sm", 2, f16)
                            for j in range(4):
                                g = h * 4 + j
                                te.transpose(tp[:, j * 128:(j + 1) * 128],
                                             src[:, g, sub * 128:(sub + 1) * 128],
                                             ident[:])
                            sc.activation(dst[:, sub, h * 512:(h + 1) * 512],
                                          tp[:], AF.Copy)
                # g1[o, d] += dh1_tm.T @ k_tm ; g2[i, o] += dp_tm.T @ a1_tm
                for go in range(GO):
                    p = _pst(ps, [128, DIM], "sm", 2)
                    for sub in range(nsub):
                        te.matmul(p[:], dh1_tm[:, sub, go * 128:(go + 1) * 128],
                                  k_tm[:, sub, :], start=(sub == 0),
                                  stop=(sub == nsub - 1))
                    ve.scalar_tensor_tensor(g1acc[:, go, :], p[:], C_LOSS,
                                            g1acc[:, go, :], ALU.mult, ALU.add)
                for gi in range(GD):
                    for h in range(2):
                        p = _pst(ps, [128, 512], "sm", 2)
                        for sub in range(nsub):
                            te.matmul(p[:],
                                      dp_tm[:, sub, gi * 128:(gi + 1) * 128],
                                      a1_tm[:, sub, h * 512:(h + 1) * 512],
                                      start=(sub == 0), stop=(sub == nsub - 1))
                        ve.scalar_tensor_tensor(g2acc[:, gi,
                                                      h * 512:(h + 1) * 512],
                                                p[:], C_LOSS,
                                                g2acc[:, gi,
                                                      h * 512:(h + 1) * 512],
                                                ALU.mult, ALU.add)

        # ---- launch collectives (overlap with pass 2) ----
        part = work.tile([128, GD, B], f32, name="part", tag="part")
        for bcol in range(B):
            ve.tensor_scalar_mul(part[:, :, bcol:bcol + 1], xsum[:],
                                 sel[:, bcol:bcol + 1])
        sy.dma_start(d["cc_x_in"][:].rearrange("p (g b) -> p g b", g=GD),
                     part[:])
        gp.collective_compute("AllReduce", ALU.add, replica_groups=REPLICA,
                              ins=[d["cc_x_in"][:]], outs=[d["cc_x_out"][:]])
        g1v = d["cc_g_in"][:, 0:SL1].rearrange("s (p d) -> p s d", p=128)
        sy.dma_start(g1v, g1acc[:])
        g2v = d["cc_g_in"][:, SL1:].rearrange(
            "(gi ph) (q o) -> ph q gi o", gi=GD, q=64)
        for ph in range(2):
            sy.dma_start(g2v[ph], g2acc[ph * 64:(ph + 1) * 64, :, :])
        gp.collective_compute("ReduceScatter", ALU.add, replica_groups=REPLICA,
                              ins=[d["cc_g_in"][:]], outs=[d["cc_g_out"][:]])
        # (work pool stays open through pass 2, closed before the tail)

        # ============ PASS 2: queries -> retrieved -> output ============
        with tc.tile_pool(name="w_q", bufs=1) as w_q:
            load_proj_weights(w_q, "q")
            woutT = w_q.tile([128, GD, DIM], f16, name="woutT")
            sy.dma_start(woutT[:], d["woutT_d"][:])
            for ci in range(NCH):
                xc = work.tile([128, GD, C + PAD], f16, name="xc", tag="xc",
                               bufs=2)
                sy.dma_start(xc[:], d["xT"][:, :, ci * C:ci * C + C + PAD])
                q_n = work.tile([128, GD, C], f16, name="q_n", tag="k_n")
                proj_conv_silu("q", xc, q_n)
                l2norm_inplace(q_n)
                aq = work.tile([128, GO, C], f16, name="aq", tag="a1")
                ph = _pst(ps, [128, GO, C], "h4", 1)
                for go in range(GO):
                    for gd in range(GD):
                        te.matmul(ph[:, go, :],
                                  mw1T[:, gd, go * 128:(go + 1) * 128],
                                  q_n[:, gd, :], start=(gd == 0),
                                  stop=(gd == GD - 1))
                sc.activation(aq[:], ph[:], AF.Silu)
                r_s = work.tile([128, GD, C], f16, name="r_s", tag="dpredu")
                pr = _pst(ps, [128, GD, C], "h4", 1)
                for gi in range(GD):
                    for go in range(GO):
                        te.matmul(pr[:, gi, :],
                                  mw2T[:, go, gi * 128:(gi + 1) * 128],
                                  aq[:, go, :], start=(go == 0),
                                  stop=(go == GO - 1))
                sc.activation(r_s[:], pr[:], AF.Copy)
                outs = work.tile([128, GD, C], f32, name="outs", tag="vs")
                po = _pst(ps, [128, GD, C], "h4", 1)
                for gu in range(GD):
                    for gi in range(GD):
                        te.matmul(po[:, gu, :],
                                  woutT[:, gi, gu * 128:(gu + 1) * 128],
                                  r_s[:, gi, :], start=(gi == 0),
                                  stop=(gi == GD - 1))
                sc.activation(outs[:], po[:], AF.Copy)
                sy.dma_start(d["outT"][:, :, ci * C:(ci + 1) * C], outs[:])

        work_cm.__exit__(None, None, None)

        # ============ tail: gates, momentum/decay update ============
        with tc.tile_pool(name="tail", bufs=1) as tail:
            gwT = tail.tile([128, GD, 3, DIM], f32, name="gwT")
            sy.dma_start(gwT[:], d["gwT_d"][:])
            gateb = tail.tile([128, GD, 3], f32, name="gateb")
            sy.dma_start(gateb[:], d["gateb_d"][:])
            xsb = tail.tile([128, GD, B], f32, name="xsb")
            sy.dma_start(xsb[:],
                         d["cc_x_out"][:].rearrange("p (g b) -> p g b", g=GD))
            gsc = tail.tile([1, 3], f32, name="gsc")
            for w in range(3):
                sig = tail.tile([128, GD, B], f32, name=f"sig{w}",
                                tag="sig")
                for gj in range(GD):
                    p = _pst(ps, [128, B], "mm", 2)
                    for gd in range(GD):
                        te.matmul(p[:],
                                  gwT[:, gd, w, gj * 128:(gj + 1) * 128],
                                  xsb[:, gd, :], start=(gd == 0),
                                  stop=(gd == GD - 1))
                    sc.activation(sig[:, gj, :], p[:], AF.Sigmoid,
                                  bias=gateb[:, gj, w:w + 1], scale=1.0 / S)
                srow = tail.tile([128, 1], f32, name=f"srow{w}", tag="srow")
                ve.tensor_reduce(srow[:], sig[:], mybir.AxisListType.XY,
                                 ALU.add)
                p = _pst(ps, [1, 1], "mm", 2)
                te.matmul(p[:], ones_col32[:], srow[:])
                sc.activation(gsc[0:1, w:w + 1], p[:], AF.Copy,
                              scale=1.0 / (DIM * B))
            ones_row32 = tail.tile([1, 128], f32, name="ones_row32")
            gp.memset(ones_row32[:], 1.0)
            gbp = _pst(ps, [128, 3], "mm", 2)
            te.matmul(gbp[:], ones_row32[:], gsc[:])
            gb = tail.tile([128, 3], f32, name="gb")
            sc.activation(gb[:], gbp[:], AF.Copy)
            oneminus = tail.tile([128, 1], f32, name="oneminus")
            ve.tensor_scalar(oneminus[:], gb[:, 0:1], -1.0, 1.0, ALU.mult,
                             ALU.add)
            theta = tail.tile([128, 1], f32, name="theta")
            ve.tensor_scalar_mul(theta[:], gb[:, 1:2], MEM_LR)
            eta = tail.tile([128, 1], f32, name="eta")
            ve.tensor_scalar_mul(eta[:], gb[:, 2:3], MEM_MOM)

            g1s = tail.tile([128, DIM], f32, name="g1s")
            sy.dma_start(g1s[:],
                         d["cc_g_out"][0:SL1].rearrange("(p d) -> p d", p=128))
            g2s = tail.tile([64, HID], f32, name="g2s")
            sy.dma_start(g2s[:],
                         d["cc_g_out"][SL1:].rearrange("(q o) -> q o", q=64))
            mw1s = tail.tile([128, DIM], f32, name="mw1s")
            sy.dma_start(mw1s[:], d["mw1s_d"][:])
            mom1s = tail.tile([128, DIM], f32, name="mom1s")
            sy.dma_start(mom1s[:], d["mom1s_d"][:])
            mw2s = tail.tile([64, HID], f32, name="mw2s")
            sy.dma_start(mw2s[:], d["mw2s_d"][:])
            mom2s = tail.tile([64, HID], f32, name="mom2s")
            sy.dma_start(mom2s[:], d["mom2s_d"][:])

            for gsl, mws, moms, s_out, nw_out, np_ in (
                    (g1s, mw1s, mom1s, d["s1_s"], d["nw1_s"], 128),
                    (g2s, mw2s, mom2s, d["s2_s"], d["nw2_s"], 64)):
                t1 = tail.tile(list(gsl.shape), f32, name="t1", tag="t1")
                ve.tensor_scalar_mul(t1[:], gsl[:], theta[0:np_, :])
                st = tail.tile(list(gsl.shape), f32, name="st", tag="st")
                ve.scalar_tensor_tensor(st[:], moms[:], eta[0:np_, :], t1[:],
                                        ALU.mult, ALU.subtract)
                nwt = tail.tile(list(gsl.shape), f32, name="nwt", tag="nwt")
                ve.scalar_tensor_tensor(nwt[:], mws[:], oneminus[0:np_, :],
                                        st[:], ALU.mult, ALU.add)
                sy.dma_start(s_out[:], st[:])
                sy.dma_start(nw_out[:], nwt[:])


# ======================= host side =======================

def _gT(w, dt=np.float16):
    """(out, in=512) torch-Linear weight -> grouped lhsT [128, 4, out]."""
    return np.ascontiguousarray(
        w.T.reshape(GD, 128, -1).transpose(1, 0, 2)).astype(dt)


def _g_natural(w, groups, dt=np.float16):
    """(rows, cols) -> grouped [128, groups, cols] (rows on partitions)."""
    return np.ascontiguousarray(
        w.reshape(groups, 128, -1).transpose(1, 0, 2)).astype(dt)


def _prep(inputs):
    x = np.asarray(inputs["x"], np.float32)
    shared = {}
    for t, (a, b_) in {"k": ("w_k1", "w_k2"), "v": ("w_v1", "w_v2"),
                       "q": ("w_q1", "w_q2")}.items():
        shared[f"w1T_{t}"] = _gT(np.asarray(inputs[a], np.float32))
        shared[f"w2T_{t}"] = _gT(np.asarray(inputs[b_], np.float32))
    mem_w1 = np.asarray(inputs["mem_w1"], np.float32)
    mem_w2 = np.asarray(inputs["mem_w2"], np.float32)
    shared["mw1T"] = _gT(mem_w1)                       # [128,4,1024]
    shared["mw2T"] = _g_natural(mem_w2.T, GO)          # [128,8,512]
    shared["mw2n"] = _g_natural(mem_w2, GD)            # [128,4,1024]
    shared["woutT"] = _gT(np.asarray(inputs["w_out"], np.float32))
    gw = np.stack([_gT(np.asarray(inputs[f"gate_{t}_w"], np.float32), np.float32)
                   for t in ("d", "lr", "m")], axis=2)  # [128,4,3,512]
    shared["gwT"] = np.ascontiguousarray(gw)
    shared["ident"] = np.eye(128, dtype=np.float16)
    shared["ones"] = np.ones((128, 129), np.float16)
    cw = np.stack([np.asarray(inputs[f"conv_{t}_w"], np.float32)[:, 0, :]
                   for t in ("k", "v", "q")], axis=1)   # (512, 3, 4)
    shared["convw"] = np.ascontiguousarray(
        cw.reshape(GD, 128, 3, K).transpose(1, 0, 2, 3))
    cb = np.stack([np.asarray(inputs[f"conv_{t}_b"], np.float32)
                   for t in ("k", "v", "q")], axis=1)   # (512, 3)
    shared["convb"] = np.ascontiguousarray(
        cb.reshape(GD, 128, 3).transpose(1, 0, 2))
    gb = np.stack([np.asarray(inputs[f"gate_{t}_b"], np.float32)
                   for t in ("d", "lr", "m")], axis=1)  # (512, 3)
    shared["gateb"] = np.ascontiguousarray(
        gb.reshape(GD, 128, 3).transpose(1, 0, 2))
    mom1 = np.asarray(inputs["mom1"], np.float32)
    mom2 = np.asarray(inputs["mom2"], np.float32)

    in_maps = []
    for c in range(N_CORES):
        b = c // 2
        half = c % 2
        seq = x[b]
        if half == 0:
            seg = np.concatenate(
                [np.zeros((PAD, DIM), np.float32), seq[0:T]], axis=0)
        else:
            seg = seq[T - PAD:2 * T]
        xTg = np.ascontiguousarray(
            seg.T.reshape(GD, 128, T + PAD).transpose(1, 0, 2)).astype(np.float16)
        sel = np.zeros((128, B), np.float32)
        sel[:, b] = 1.0
        m = dict(shared)
        m["xT"] = xTg
        m["sel"] = sel
        m["mw1s"] = np.ascontiguousarray(mem_w1[c * 128:(c + 1) * 128])
        m["mom1s"] = np.ascontiguousarray(mom1[c * 128:(c + 1) * 128])
        m["mw2s"] = np.ascontiguousarray(mem_w2[c * 64:(c + 1) * 64])
        m["mom2s"] = np.ascontiguousarray(mom2[c * 64:(c + 1) * 64])
        in_maps.append(m)
    return in_maps


def _unshard(results):
    outs = []
    for c in range(N_CORES):
        a = results[c]["outT"]          # [128, 4, 4096]
        outs.append(np.moveaxis(a, 1, 0).reshape(DIM, T).T)
    output = np.concatenate(outs, axis=0).reshape(B, S, DIM)
    nw1 = np.concatenate([results[c]["nw1_s"] for c in range(N_CORES)], axis=0)
    s1 = np.concatenate([results[c]["s1_s"] for c in range(N_CORES)], axis=0)
    nw2 = np.concatenate([results[c]["nw2_s"] for c in range(N_CORES)], axis=0)
    s2 = np.concatenate([results[c]["s2_s"] for c in range(N_CORES)], axis=0)
    return (output, nw1, nw2, s1, s2)


def run(in_maps, trace=False):
    if "nc" not in _CACHE:
        _CACHE["nc"] = _build()
    return run_bass_kernel_spmd(_CACHE["nc"], in_maps,
                                core_ids=list(range(N_CORES)), trace=trace)


def kernel(**inputs):
    res = run(_prep(inputs))
    return _unshard(res.results)


if __name__ == "__main__":
    import jax
    import reference as R
    inp = {k: np.asarray(v) for k, v in R.setup_inputs().items()}
    got = kernel(**inp)
    print([g.shape for g in got])


# revision 12
# speedup vs baseline: 1.0460x; 1.0407x over previous
"""Trainium2 Bass kernel for nn_NeuralLongTermMemory.

Sharding: tokens (B*S = 32768) split 8 ways -> 4096 tokens/core (half a
batch-sequence each).  All weights replicated.  Gradients of the memory
MLP are partial-summed per core and combined with one ReduceScatter; the
data-dependent scalar gates use one small AllReduce of per-batch x sums.

Layout: feature-major ("transposed") everywhere - features on SBUF
partitions (grouped [128, G, cols]), tokens on the free dimension.  The
causal depthwise conv then becomes shifted-window ops on the free dim.
The gradient outer-products need token-major operands; those are
produced with PE transposes.

All matmuls run in float32r (full-rate fp32, ~1e-4 rel rounding).
"""

import numpy as np
import concourse.bass as bass
import concourse.mybir as mybir
import concourse.tile as tile
from concourse import bacc
from concourse.bass_utils import run_bass_kernel_spmd

f32 = mybir.dt.float32
f32r = mybir.dt.float32r
f16 = mybir.dt.float16
AF = mybir.ActivationFunctionType
ALU = mybir.AluOpType

B, S, DIM, HID, K = 4, 8192, 512, 1024, 4
N_CORES = 8
T = B * S // N_CORES        # 4096 tokens per core
HALO = K - 1                # 3
PAD = 4                     # input halo columns (even matmul free dims)
C = 256                     # chunk tokens
NCH = T // C
GD = DIM // 128             # 4
GO = HID // 128             # 8
C_LOSS = 2.0 / (B * S * DIM)
MEM_LR, MEM_MOM = 0.01, 0.9
EPS = 1e-12
SL1 = 128 * 512             # g1 slice elems per core
REPLICA = [list(range(N_CORES))]

_CACHE = {}


def _pst(ps, shape, tag, bufs, dt=f32):
    return ps.tile(shape, dt, name=tag, tag=tag, bufs=bufs, space="PSUM")


def _build():
    nc = bacc.Bacc("TRN2", target_bir_lowering=False, debug=False,
                   num_devices=N_CORES)

    def din(name, shape, dt=f16):
        return nc.dram_tensor(name, shape, dt, kind="ExternalInput")

    def dout(name, shape):
        return nc.dram_tensor(name, shape, f32, kind="ExternalOutput")

    xT = din("xT", [128, GD, T + PAD])
    w1T = {t: din(f"w1T_{t}", [128, GD, DIM]) for t in "kvq"}
    w2T = {t: din(f"w2T_{t}", [128, GD, DIM]) for t in "kvq"}
    mw1T_d = din("mw1T", [128, GD, HID])
    mw2T_d = din("mw2T", [128, GO, DIM])
    mw2n_d = din("mw2n", [128, GD, HID])
    woutT_d = din("woutT", [128, GD, DIM])
    gwT_d = din("gwT", [128, GD, 3, DIM], f32)
    ident_d = din("ident", [128, 128])
    ones_d = din("ones", [128, 129])
    convw_d = din("convw", [128, GD, 3, K], f32)
    convb_d = din("convb", [128, GD, 3], f32)
    gateb_d = din("gateb", [128, GD, 3], f32)
    sel_d = din("sel", [128, B], f32)
    mw1s_d = din("mw1s", [128, DIM], f32)
    mom1s_d = din("mom1s", [128, DIM], f32)
    mw2s_d = din("mw2s", [64, HID], f32)
    mom2s_d = din("mom2s", [64, HID], f32)

    outT = dout("outT", [128, GD, T])
    nw1_s = dout("nw1_s", [128, DIM])
    s1_s = dout("s1_s", [128, DIM])
    nw2_s = dout("nw2_s", [64, HID])
    s2_s = dout("s2_s", [64, HID])

    # collective scratch
    cc_g_in = nc.dram_tensor("cc_g_in", [N_CORES, 2 * SL1], f32)
    cc_g_out = nc.dram_tensor("cc_g_out", [2 * SL1], f32)
    cc_x_in = nc.dram_tensor("cc_x_in", [128, GD * B], f32)
    cc_x_out = nc.dram_tensor("cc_x_out", [128, GD * B], f32,
                              addr_space="Shared")

    with tile.TileContext(nc) as tc:
        _emit(nc, tc, locals())
    nc.compile()
    return nc


def _emit(nc, tc, d):
    sc, ve, te, sy, gp = nc.scalar, nc.vector, nc.tensor, nc.sync, nc.gpsimd

    with (
        tc.tile_pool(name="consts", bufs=1) as consts,
        tc.tile_pool(name="accs", bufs=1) as accs,
        tc.tile_pool(name="wmem", bufs=1) as wmem,
        tc.tile_pool(name="ps", bufs=1, space="PSUM") as ps,
    ):
        # ---- constants / resident weights ----
        ident = consts.tile([128, 128], f16)
        sy.dma_start(ident[:], d["ident_d"][:])
        onesb = consts.tile([128, 129], f16)
        sy.dma_start(onesb[:], d["ones_d"][:])
        ones_col = onesb[:, 0:1]
        ones_row = onesb[0:1, 1:129]
        ones_col32 = consts.tile([128, 1], f32)
        gp.memset(ones_col32[:], 1.0)
        convw = consts.tile([128, GD, 3, K], f32)
        sy.dma_start(convw[:], d["convw_d"][:])
        convb = consts.tile([128, GD, 3], f32)
        sy.dma_start(convb[:], d["convb_d"][:])
        sel = consts.tile([128, B], f32)
        sy.dma_start(sel[:], d["sel_d"][:])

        mw1T = wmem.tile([128, GD, HID], f16)
        sy.dma_start(mw1T[:], d["mw1T_d"][:])
        mw2T = wmem.tile([128, GO, DIM], f16)
        sy.dma_start(mw2T[:], d["mw2T_d"][:])
        mw2n = wmem.tile([128, GD, HID], f16)
        sy.dma_start(mw2n[:], d["mw2n_d"][:])

        g1acc = accs.tile([128, GO, DIM], f32)
        gp.memset(g1acc[:], 0.0)
        g2acc = accs.tile([128, GD, HID], f32)
        gp.memset(g2acc[:], 0.0)
        xsum = accs.tile([128, GD, 1], f32)
        gp.memset(xsum[:], 0.0)

        w1sb = {}
        w2sb = {}

        def load_proj_weights(pool, tensors):
            for t in tensors:
                wa = pool.tile([128, GD, DIM], f16, name=f"w1sb_{t}",
                               tag=f"w1sb_{t}")
                sy.dma_start(wa[:], d["w1T"][t][:])
                w1sb[t] = wa
                wb = pool.tile([128, GD, DIM], f16, name=f"w2sb_{t}",
                               tag=f"w2sb_{t}")
                sy.dma_start(wb[:], d["w2T"][t][:])
                w2sb[t] = wb

        def proj_conv_silu(tn, xc, out_t):
            ncols = C + PAD
            y1s = work.tile([128, GD, ncols], f16, name="y1s", tag="y1s",
                            bufs=2)
            for gj in range(GD):
                p = _pst(ps, [128, ncols], "mm", 2)
                for gd in range(GD):
                    te.matmul(p[:], w1sb[tn][:, gd, gj * 128:(gj + 1) * 128],
                              xc[:, gd, :], start=(gd == 0), stop=(gd == GD - 1))
                sc.activation(y1s[:, gj, :], p[:], AF.Silu)
            ca = work.tile([128, GD, C], f32, name="ca", tag="ca", bufs=2)
            ti = "kvq".index(tn)
            for gj in range(GD):
                p = _pst(ps, [128, ncols], "mm", 2)
                for gd in range(GD):
                    te.matmul(p[:], w2sb[tn][:, gd, gj * 128:(gj + 1) * 128],
                              y1s[:, gd, :], start=(gd == 0), stop=(gd == GD - 1))
                ve.tensor_scalar(ca[:, gj, :], p[:, 1:1 + C],
                                 convw[:, gj, ti, 0:1], None, ALU.mult)
                for kk in range(1, K):
                    ve.scalar_tensor_tensor(ca[:, gj, :], p[:, 1 + kk:1 + kk + C],
                                            convw[:, gj, ti, kk:kk + 1],
                                            ca[:, gj, :], ALU.mult, ALU.add)
            for gj in range(GD):
                sc.activation(out_t[:, gj, :], ca[:, gj, :], AF.Silu,
                              bias=convb[:, gj, ti:ti + 1])

        def l2norm_inplace(src):
            """src: [128, GD, C] fp32r silu output; normalized in place."""
            sq = work.tile([128, GD, C], f16, name="sq", tag="sq", bufs=2)
            gp.tensor_mul(sq[:], src[:], src[:])
            ssp = _pst(ps, [1, C], "mm", 2)
            for gd in range(GD):
                te.matmul(ssp[:], ones_col, sq[:, gd, :],
                          start=(gd == 0), stop=(gd == GD - 1))
            rn = work.tile([1, C], f32, name="rn", tag="rn", bufs=2)
            sc.activation(rn[:], ssp[:], AF.Sqrt)
            ve.tensor_scalar_max(rn[:], rn[:], EPS)
            with nc.allow_low_precision("norm factor to f16"):
                rnr = work.tile([1, C], f16, name="rnr", tag="rnr", bufs=2)
                ve.reciprocal(rnr[:], rn[:])
            rnb = work.tile([128, C], f16, name="rnb", tag="rnb", bufs=2)
            gp.partition_broadcast(rnb[:], rnr[:])
            for gd in range(GD):
                ve.tensor_mul(src[:, gd, :], src[:, gd, :], rnb[:])

        # ============ PASS 1: k/v + gradient accumulation ============
        work_cm = tc.tile_pool(name="work", bufs=1)
        work = work_cm.__enter__()
        with tc.tile_pool(name="w_kv", bufs=1) as w_kv:
            load_proj_weights(w_kv, "kv")
            for ci in range(NCH):
                xc = work.tile([128, GD, C + PAD], f16, name="xc", tag="xc",
                               bufs=2)
                sy.dma_start(xc[:], d["xT"][:, :, ci * C:ci * C + C + PAD])
                # gate statistics: sum x over this chunk's tokens
                xs_c = work.tile([128, GD, 1], f32, name="xs_c", tag="xs_c")
                ve.tensor_reduce(xs_c[:], xc[:, :, PAD:], mybir.AxisListType.X,
                                 ALU.add)
                ve.tensor_add(xsum[:], xsum[:], xs_c[:])

                k_n = work.tile([128, GD, C], f16, name="k_n", tag="k_n", bufs=2)
                proj_conv_silu("k", xc, k_n)
                vs = work.tile([128, GD, C], f16, name="vs", tag="vs", bufs=2)
                proj_conv_silu("v", xc, vs)
                l2norm_inplace(k_n)

                # forward through memory MLP (feature-major)
                a1 = work.tile([128, GO, C], f16, name="a1", tag="a1", bufs=2)
                sp1 = work.tile([128, GO, C], f16, name="sp1", tag="sp1", bufs=2)
                for hh in range(2):
                    ph = _pst(ps, [128, GD, C], "h2", 2)
                    for gg in range(GD):
                        go = hh * GD + gg
                        for gd in range(GD):
                            te.matmul(ph[:, gg, :],
                                      mw1T[:, gd, go * 128:(go + 1) * 128],
                                      k_n[:, gd, :], start=(gd == 0),
                                      stop=(gd == GD - 1))
                    sc.activation(a1[:, hh * GD:(hh + 1) * GD, :], ph[:],
                                  AF.Silu)
                    sc.activation(sp1[:, hh * GD:(hh + 1) * GD, :], ph[:],
                                  AF.Derivative_silu)
                dpredu = work.tile([128, GD, C], f16, name="dpredu",
                                   tag="dpredu", bufs=2)
                pp = _pst(ps, [128, GD, C], "h2", 2)
                for gi in range(GD):
                    for go in range(GO):
                        te.matmul(pp[:, gi, :],
                                  mw2T[:, go, gi * 128:(gi + 1) * 128],
                                  a1[:, go, :], start=(go == 0),
                                  stop=(go == GO - 1))
                ve.tensor_sub(dpredu[:], pp[:], vs[:])
                dh1 = work.tile([128, GO, C], f16, name="dh1", tag="dh1", bufs=2)
                for hh in range(2):
                    pd = _pst(ps, [128, GD, C], "h2", 2)
                    for gg in range(GD):
                        go = hh * GD + gg
                        for gi in range(GD):
                            te.matmul(pd[:, gg, :],
                                      mw2n[:, gi, go * 128:(go + 1) * 128],
                                      dpredu[:, gi, :], start=(gi == 0),
                                      stop=(gi == GD - 1))
                    ve.tensor_mul(dh1[:, hh * GD:(hh + 1) * GD, :], pd[:],
                                  sp1[:, hh * GD:(hh + 1) * GD, :])

                # transpose to token-major for the outer products
                nsub = C // 128
                k_tm = work.tile([128, nsub, DIM], f16, name="k_tm",
                                 tag="k_tm", bufs=2)
                dp_tm = work.tile([128, nsub, DIM], f16, name="dp_tm",
                                  tag="dp_tm", bufs=2)
                a1_tm = work.tile([128, nsub, HID], f16, name="a1_tm",
                                  tag="a1_tm", bufs=2)
                dh1_tm = work.tile([128, nsub, HID], f16, name="dh1_tm",
                                   tag="dh1_tm", bufs=2)
                for sub in range(nsub):
                    for src, dst, ng in ((k_n, k_tm, GD), (dpredu, dp_tm, GD),
                                         (a1, a1_tm, GO), (dh1, dh1_tm, GO)):
                        for h in range(ng // 4):
                            tp = _pst(ps, [128, 512], "sm", 2, f16)
                            for j in range(4):
                                g = h * 4 + j
                                te.transpose(tp[:, j * 128:(j + 1) * 128],
                                             src[:, g, sub * 128:(sub + 1) * 128],
                                             ident[:])
                            sc.activation(dst[:, sub, h * 512:(h + 1) * 512],
                                          tp[:], AF.Copy)
                # g1[o, d] += dh1_tm.T @ k_tm ; g2[i, o] += dp_tm.T @ a1_tm
                for go in range(GO):
                    p = _pst(ps, [128, DIM], "sm", 2)
                    for sub in range(nsub):
                        te.matmul(p[:], dh1_tm[:, sub, go * 128:(go + 1) * 128],
                                  k_tm[:, sub, :], start=(sub == 0),
                                  stop=(sub == nsub - 1))
                    ve.scalar_tensor_tensor(g1acc[:, go, :], p[:], C_LOSS,
                                            g1acc[:, go, :], ALU.mult, ALU.add)
                for gi in range(GD):
                    for h in range(2):
                        p = _pst(ps, [128, 512], "sm", 2)
                        for sub in range(nsub):
                            te.matmul(p[:],
                                      dp_tm[:, sub, gi * 128:(gi + 1) * 128],
                                      a1_tm[:, sub, h * 512:(h + 1) * 512],
                                      start=(sub == 0), stop=(sub == nsub - 1))
                        ve.scalar_tensor_tensor(g2acc[:, gi,
                                                      h * 512:(h + 1) * 512],
                                                p[:], C_LOSS,
                                                g2acc[:, gi,
                                                      h * 512:(h + 1) * 512],
                                                ALU.mult, ALU.add)

        # ---- launch collectives (overlap with pass 2) ----
        part = work.tile([128, GD, B], f32, name="part", tag="part")
        for bcol in range(B):
            ve.tensor_scalar_mul(part[:, :, bcol:bcol + 1], xsum[:],
                                 sel[:, bcol:bcol + 1])
        sy.dma_start(d["cc_x_in"][:].rearrange("p (g b) -> p g b", g=GD),
                     part[:])
        gp.collective_compute("AllReduce", ALU.add, replica_groups=REPLICA,
                              ins=[d["cc_x_in"][:]], outs=[d["cc_x_out"][:]])
        g1v = d["cc_g_in"][:, 0:SL1].rearrange("s (p d) -> p s d", p=128)
        sy.dma_start(g1v, g1acc[:])
        g2v = d["cc_g_in"][:, SL1:].rearrange(
            "(gi ph) (q o) -> ph q gi o", gi=GD, q=64)
        for ph in range(2):
            sy.dma_start(g2v[ph], g2acc[ph * 64:(ph + 1) * 64, :, :])
        gp.collective_compute("ReduceScatter", ALU.add, replica_groups=REPLICA,
                              ins=[d["cc_g_in"][:]], outs=[d["cc_g_out"][:]])
        # (work pool stays open through pass 2, closed before the tail)

        # ============ PASS 2: queries -> retrieved -> output ============
        with tc.tile_pool(name="w_q", bufs=1) as w_q:
            load_proj_weights(w_q, "q")
            woutT = w_q.tile([128, GD, DIM], f16, name="woutT")
            sy.dma_start(woutT[:], d["woutT_d"][:])
            for ci in range(NCH):
                xc = work.tile([128, GD, C + PAD], f16, name="xc", tag="xc",
                               bufs=2)
                sy.dma_start(xc[:], d["xT"][:, :, ci * C:ci * C + C + PAD])
                q_n = work.tile([128, GD, C], f16, name="q_n", tag="k_n", bufs=2)
                proj_conv_silu("q", xc, q_n)
                l2norm_inplace(q_n)
                aq = work.tile([128, GO, C], f16, name="aq", tag="a1", bufs=2)
                for hh in range(2):
                    ph = _pst(ps, [128, GD, C], "h2", 2)
                    for gg in range(GD):
                        go = hh * GD + gg
                        for gd in range(GD):
                            te.matmul(ph[:, gg, :],
                                      mw1T[:, gd, go * 128:(go + 1) * 128],
                                      q_n[:, gd, :], start=(gd == 0),
                                      stop=(gd == GD - 1))
                    sc.activation(aq[:, hh * GD:(hh + 1) * GD, :], ph[:],
                                  AF.Silu)
                r_s = work.tile([128, GD, C], f16, name="r_s", tag="dpredu", bufs=2)
                pr = _pst(ps, [128, GD, C], "h2", 2)
                for gi in range(GD):
                    for go in range(GO):
                        te.matmul(pr[:, gi, :],
                                  mw2T[:, go, gi * 128:(gi + 1) * 128],
                                  aq[:, go, :], start=(go == 0),
                                  stop=(go == GO - 1))
                sc.activation(r_s[:], pr[:], AF.Copy)
                outs = work.tile([128, GD, C], f32, name="outs", tag="outs", bufs=2)
                po = _pst(ps, [128, GD, C], "h2", 2)
                for gu in range(GD):
                    for gi in range(GD):
                        te.matmul(po[:, gu, :],
                                  woutT[:, gi, gu * 128:(gu + 1) * 128],
                                  r_s[:, gi, :], start=(gi == 0),
                                  stop=(gi == GD - 1))
                sc.activation(outs[:], po[:], AF.Copy)
                sy.dma_start(d["outT"][:, :, ci * C:(ci + 1) * C], outs[:])

        work_cm.__exit__(None, None, None)

        # ============ tail: gates, momentum/decay update ============
        with tc.tile_pool(name="tail", bufs=1) as tail:
            gwT = tail.tile([128, GD, 3, DIM], f32, name="gwT")
            sy.dma_start(gwT[:], d["gwT_d"][:])
            gateb = tail.tile([128, GD, 3], f32, name="gateb")
            sy.dma_start(gateb[:], d["gateb_d"][:])
            xsb = tail.tile([128, GD, B], f32, name="xsb")
            sy.dma_start(xsb[:],
                         d["cc_x_out"][:].rearrange("p (g b) -> p g b", g=GD))
            gsc = tail.tile([1, 3], f32, name="gsc")
            for w in range(3):
                sig = tail.tile([128, GD, B], f32, name=f"sig{w}",
                                tag="sig")
                for gj in range(GD):
                    p = _pst(ps, [128, B], "mm", 2)
                    for gd in range(GD):
                        te.matmul(p[:],
                                  gwT[:, gd, w, gj * 128:(gj + 1) * 128],
                                  xsb[:, gd, :], start=(gd == 0),
                                  stop=(gd == GD - 1))
                    sc.activation(sig[:, gj, :], p[:], AF.Sigmoid,
                                  bias=gateb[:, gj, w:w + 1], scale=1.0 / S)
                srow = tail.tile([128, 1], f32, name=f"srow{w}", tag="srow")
                ve.tensor_reduce(srow[:], sig[:], mybir.AxisListType.XY,
                                 ALU.add)
                p = _pst(ps, [1, 1], "mm", 2)
                te.matmul(p[:], ones_col32[:], srow[:])
                sc.activation(gsc[0:1, w:w + 1], p[:], AF.Copy,
                              scale=1.0 / (DIM * B))
            ones_row32 = tail.tile([1, 128], f32, name="ones_row32")
            gp.memset(ones_row32[:], 1.0)
            gbp = _pst(ps, [128, 3], "mm", 2)
            te.matmul(gbp[:], ones_row32[:], gsc[:])
            gb = tail.tile([128, 3], f32, name="gb")
            sc.activation(gb[:], gbp[:], AF.Copy)
            oneminus = tail.tile([128, 1], f32, name="oneminus")
            ve.tensor_scalar(oneminus[:], gb[:, 0:1], -1.0, 1.0, ALU.mult,
                             ALU.add)
            theta = tail.tile([128, 1], f32, name="theta")
            ve.tensor_scalar_mul(theta[:], gb[:, 1:2], MEM_LR)
            eta = tail.tile([128, 1], f32, name="eta")
            ve.tensor_scalar_mul(eta[:], gb[:, 2:3], MEM_MOM)

            g1s = tail.tile([128, DIM], f32, name="g1s")
            sy.dma_start(g1s[:],
                         d["cc_g_out"][0:SL1].rearrange("(p d) -> p d", p=128))
            g2s = tail.tile([64, HID], f32, name="g2s")
            sy.dma_start(g2s[:],
                         d["cc_g_out"][SL1:].rearrange("(q o) -> q o", q=64))
            mw1s = tail.tile([128, DIM], f32, name="mw1s")
            sy.dma_start(mw1s[:], d["mw1s_d"][:])
            mom1s = tail.tile([128, DIM], f32, name="mom1s")
            sy.dma_start(mom1s[:], d["mom1s_d"][:])
            mw2s = tail.tile([64, HID], f32, name="mw2s")
            sy.dma_start(mw2s[:], d["mw2s_d"][:])
            mom2s = tail.tile([64, HID], f32, name="mom2s")
            sy.dma_start(mom2s[:], d["mom2s_d"][:])

            for gsl, mws, moms, s_out, nw_out, np_ in (
                    (g1s, mw1s, mom1s, d["s1_s"], d["nw1_s"], 128),
                    (g2s, mw2s, mom2s, d["s2_s"], d["nw2_s"], 64)):
                t1 = tail.tile(list(gsl.shape), f32, name="t1", tag="t1")
                ve.tensor_scalar_mul(t1[:], gsl[:], theta[0:np_, :])
                st = tail.tile(list(gsl.shape), f32, name="st", tag="st")
                ve.scalar_tensor_tensor(st[:], moms[:], eta[0:np_, :], t1[:],
                                        ALU.mult, ALU.subtract)
                nwt = tail.tile(list(gsl.shape), f32, name="nwt", tag="nwt")
                ve.scalar_tensor_tensor(nwt[:], mws[:], oneminus[0:np_, :],
                                        st[:], ALU.mult, ALU.add)
                sy.dma_start(s_out[:], st[:])
                sy.dma_start(nw_out[:], nwt[:])


# ======================= host side =======================

def _gT(w, dt=np.float16):
    """(out, in=512) torch-Linear weight -> grouped lhsT [128, 4, out]."""
    return np.ascontiguousarray(
        w.T.reshape(GD, 128, -1).transpose(1, 0, 2)).astype(dt)


def _g_natural(w, groups, dt=np.float16):
    """(rows, cols) -> grouped [128, groups, cols] (rows on partitions)."""
    return np.ascontiguousarray(
        w.reshape(groups, 128, -1).transpose(1, 0, 2)).astype(dt)


def _prep(inputs):
    x = np.asarray(inputs["x"], np.float32)
    shared = {}
    for t, (a, b_) in {"k": ("w_k1", "w_k2"), "v": ("w_v1", "w_v2"),
                       "q": ("w_q1", "w_q2")}.items():
        shared[f"w1T_{t}"] = _gT(np.asarray(inputs[a], np.float32))
        shared[f"w2T_{t}"] = _gT(np.asarray(inputs[b_], np.float32))
    mem_w1 = np.asarray(inputs["mem_w1"], np.float32)
    mem_w2 = np.asarray(inputs["mem_w2"], np.float32)
    shared["mw1T"] = _gT(mem_w1)                       # [128,4,1024]
    shared["mw2T"] = _g_natural(mem_w2.T, GO)          # [128,8,512]
    shared["mw2n"] = _g_natural(mem_w2, GD)            # [128,4,1024]
    shared["woutT"] = _gT(np.asarray(inputs["w_out"], np.float32))
    gw = np.stack([_gT(np.asarray(inputs[f"gate_{t}_w"], np.float32), np.float32)
                   for t in ("d", "lr", "m")], axis=2)  # [128,4,3,512]
    shared["gwT"] = np.ascontiguousarray(gw)
    shared["ident"] = np.eye(128, dtype=np.float16)
    shared["ones"] = np.ones((128, 129), np.float16)
    cw = np.stack([np.asarray(inputs[f"conv_{t}_w"], np.float32)[:, 0, :]
                   for t in ("k", "v", "q")], axis=1)   # (512, 3, 4)
    shared["convw"] = np.ascontiguousarray(
        cw.reshape(GD, 128, 3, K).transpose(1, 0, 2, 3))
    cb = np.stack([np.asarray(inputs[f"conv_{t}_b"], np.float32)
                   for t in ("k", "v", "q")], axis=1)   # (512, 3)
    shared["convb"] = np.ascontiguousarray(
        cb.reshape(GD, 128, 3).transpose(1, 0, 2))
    gb = np.stack([np.asarray(inputs[f"gate_{t}_b"], np.float32)
                   for t in ("d", "lr", "m")], axis=1)  # (512, 3)
    shared["gateb"] = np.ascontiguousarray(
        gb.reshape(GD, 128, 3).transpose(1, 0, 2))
    mom1 = np.asarray(inputs["mom1"], np.float32)
    mom2 = np.asarray(inputs["mom2"], np.float32)

    in_maps = []
    for c in range(N_CORES):
        b = c // 2
        half = c % 2
        seq = x[b]
        if half == 0:
            seg = np.concatenate(
                [np.zeros((PAD, DIM), np.float32), seq[0:T]], axis=0)
        else:
            seg = seq[T - PAD:2 * T]
        xTg = np.ascontiguousarray(
            seg.T.reshape(GD, 128, T + PAD).transpose(1, 0, 2)).astype(np.float16)
        sel = np.zeros((128, B), np.float32)
        sel[:, b] = 1.0
        m = dict(shared)
        m["xT"] = xTg
        m["sel"] = sel
        m["mw1s"] = np.ascontiguousarray(mem_w1[c * 128:(c + 1) * 128])
        m["mom1s"] = np.ascontiguousarray(mom1[c * 128:(c + 1) * 128])
        m["mw2s"] = np.ascontiguousarray(mem_w2[c * 64:(c + 1) * 64])
        m["mom2s"] = np.ascontiguousarray(mom2[c * 64:(c + 1) * 64])
        in_maps.append(m)
    return in_maps


def _unshard(results):
    outs = []
    for c in range(N_CORES):
        a = results[c]["outT"]          # [128, 4, 4096]
        outs.append(np.moveaxis(a, 1, 0).reshape(DIM, T).T)
    output = np.concatenate(outs, axis=0).reshape(B, S, DIM)
    nw1 = np.concatenate([results[c]["nw1_s"] for c in range(N_CORES)], axis=0)
    s1 = np.concatenate([results[c]["s1_s"] for c in range(N_CORES)], axis=0)
    nw2 = np.concatenate([results[c]["nw2_s"] for c in range(N_CORES)], axis=0)
    s2 = np.concatenate([results[c]["s2_s"] for c in range(N_CORES)], axis=0)
    return (output, nw1, nw2, s1, s2)


def run(in_maps, trace=False):
    if "nc" not in _CACHE:
        _CACHE["nc"] = _build()
    return run_bass_kernel_spmd(_CACHE["nc"], in_maps,
                                core_ids=list(range(N_CORES)), trace=trace)


def kernel(**inputs):
    res = run(_prep(inputs))
    return _unshard(res.results)


if __name__ == "__main__":
    import jax
    import reference as R
    inp = {k: np.asarray(v) for k, v in R.setup_inputs().items()}
    got = kernel(**inp)
    print([g.shape for g in got])


# revision 13
# speedup vs baseline: 1.1065x; 1.0578x over previous
"""Trainium2 Bass kernel for nn_NeuralLongTermMemory.

Sharding: tokens (B*S = 32768) split 8 ways -> 4096 tokens/core (half a
batch-sequence each).  All weights replicated.  Gradients of the memory
MLP are partial-summed per core and combined with one ReduceScatter; the
data-dependent scalar gates use one small AllReduce of per-batch x sums.

Layout: feature-major ("transposed") everywhere - features on SBUF
partitions (grouped [128, G, cols]), tokens on the free dimension.  The
causal depthwise conv then becomes shifted-window ops on the free dim.
The gradient outer-products need token-major operands; those are
produced with PE transposes.

All matmuls run in float32r (full-rate fp32, ~1e-4 rel rounding).
"""

import numpy as np
import concourse.bass as bass
import concourse.mybir as mybir
import concourse.tile as tile
from concourse import bacc
from concourse.bass_utils import run_bass_kernel_spmd

f32 = mybir.dt.float32
f32r = mybir.dt.float32r
f16 = mybir.dt.float16
AF = mybir.ActivationFunctionType
ALU = mybir.AluOpType

B, S, DIM, HID, K = 4, 8192, 512, 1024, 4
N_CORES = 8
T = B * S // N_CORES        # 4096 tokens per core
HALO = K - 1                # 3
PAD = 4                     # input halo columns (even matmul free dims)
C = 256                     # chunk tokens
NCH = T // C
GD = DIM // 128             # 4
GO = HID // 128             # 8
C_LOSS = 2.0 / (B * S * DIM)
MEM_LR, MEM_MOM = 0.01, 0.9
EPS = 1e-12
SL1 = 128 * 512             # g1 slice elems per core
REPLICA = [list(range(N_CORES))]

_CACHE = {}


def _pst(ps, shape, tag, bufs, dt=f32):
    return ps.tile(shape, dt, name=tag, tag=tag, bufs=bufs, space="PSUM")


def _build():
    nc = bacc.Bacc("TRN2", target_bir_lowering=False, debug=False,
                   num_devices=N_CORES)

    def din(name, shape, dt=f16):
        return nc.dram_tensor(name, shape, dt, kind="ExternalInput")

    def dout(name, shape):
        return nc.dram_tensor(name, shape, f32, kind="ExternalOutput")

    xT = din("xT", [128, GD, T + PAD])
    w1T = {t: din(f"w1T_{t}", [128, GD, DIM]) for t in "kvq"}
    w2T = {t: din(f"w2T_{t}", [128, GD, DIM]) for t in "kvq"}
    mw1T_d = din("mw1T", [128, GD, HID])
    mw2T_d = din("mw2T", [128, GO, DIM])
    mw2n_d = din("mw2n", [128, GD, HID])
    woutT_d = din("woutT", [128, GD, DIM])
    gwT_d = din("gwT", [128, GD, 3, DIM], f32)
    ident_d = din("ident", [128, 128])
    ones_d = din("ones", [128, 129])
    convw_d = din("convw", [128, GD, 3, K], f32)
    convb_d = din("convb", [128, GD, 3], f32)
    gateb_d = din("gateb", [128, GD, 3], f32)
    sel_d = din("sel", [128, B], f32)
    mw1s_d = din("mw1s", [128, DIM], f32)
    mom1s_d = din("mom1s", [128, DIM], f32)
    mw2s_d = din("mw2s", [64, HID], f32)
    mom2s_d = din("mom2s", [64, HID], f32)

    outT = dout("outT", [128, GD, T])
    nw1_s = dout("nw1_s", [128, DIM])
    s1_s = dout("s1_s", [128, DIM])
    nw2_s = dout("nw2_s", [64, HID])
    s2_s = dout("s2_s", [64, HID])

    # collective scratch
    cc_g_in = nc.dram_tensor("cc_g_in", [N_CORES, 2 * SL1], f32)
    cc_g_out = nc.dram_tensor("cc_g_out", [2 * SL1], f32)
    cc_x_in = nc.dram_tensor("cc_x_in", [128, GD * B], f32)
    cc_x_out = nc.dram_tensor("cc_x_out", [128, GD * B], f32,
                              addr_space="Shared")

    with tile.TileContext(nc) as tc:
        _emit(nc, tc, locals())
    nc.compile()
    return nc


def _emit(nc, tc, d):
    sc, ve, te, sy, gp = nc.scalar, nc.vector, nc.tensor, nc.sync, nc.gpsimd

    with (
        tc.tile_pool(name="consts", bufs=1) as consts,
        tc.tile_pool(name="accs", bufs=1) as accs,
        tc.tile_pool(name="wmem", bufs=1) as wmem,
        tc.tile_pool(name="ps", bufs=1, space="PSUM") as ps,
    ):
        # ---- constants / resident weights ----
        ident = consts.tile([128, 128], f16)
        sy.dma_start(ident[:], d["ident_d"][:])
        onesb = consts.tile([128, 129], f16)
        sy.dma_start(onesb[:], d["ones_d"][:])
        ones_col = onesb[:, 0:1]
        ones_row = onesb[0:1, 1:129]
        ones_col32 = consts.tile([128, 1], f32)
        gp.memset(ones_col32[:], 1.0)
        convw = consts.tile([128, GD, 3, K], f32)
        sy.dma_start(convw[:], d["convw_d"][:])
        convb = consts.tile([128, GD, 3], f32)
        sy.dma_start(convb[:], d["convb_d"][:])
        sel = consts.tile([128, B], f32)
        sy.dma_start(sel[:], d["sel_d"][:])

        mw1T = wmem.tile([128, GD, HID], f16)
        sy.dma_start(mw1T[:], d["mw1T_d"][:])
        mw2T = wmem.tile([128, GO, DIM], f16)
        sy.dma_start(mw2T[:], d["mw2T_d"][:])
        mw2n = wmem.tile([128, GD, HID], f16)
        sy.dma_start(mw2n[:], d["mw2n_d"][:])

        g1acc = accs.tile([128, GO, DIM], f32)
        gp.memset(g1acc[:], 0.0)
        g2acc = accs.tile([128, GD, HID], f32)
        gp.memset(g2acc[:], 0.0)
        xsum = accs.tile([128, GD, 1], f32)
        gp.memset(xsum[:], 0.0)

        w1sb = {}
        w2sb = {}

        def load_proj_weights(pool, tensors):
            for t in tensors:
                wa = pool.tile([128, GD, DIM], f16, name=f"w1sb_{t}",
                               tag=f"w1sb_{t}")
                sy.dma_start(wa[:], d["w1T"][t][:])
                w1sb[t] = wa
                wb = pool.tile([128, GD, DIM], f16, name=f"w2sb_{t}",
                               tag=f"w2sb_{t}")
                sy.dma_start(wb[:], d["w2T"][t][:])
                w2sb[t] = wb

        def proj_conv_silu(tn, xc, out_t):
            ncols = C + PAD
            y1s = work.tile([128, GD, ncols], f16, name="y1s", tag="y1s",
                            bufs=2)
            for gj in range(GD):
                p = _pst(ps, [128, ncols], "mm", 2)
                for gd in range(GD):
                    te.matmul(p[:], w1sb[tn][:, gd, gj * 128:(gj + 1) * 128],
                              xc[:, gd, :], start=(gd == 0), stop=(gd == GD - 1))
                sc.activation(y1s[:, gj, :], p[:], AF.Silu)
            ca = work.tile([128, GD, C], f32, name="ca", tag="ca", bufs=2)
            ti = "kvq".index(tn)
            for gj in range(GD):
                p = _pst(ps, [128, ncols], "mm", 2)
                for gd in range(GD):
                    te.matmul(p[:], w2sb[tn][:, gd, gj * 128:(gj + 1) * 128],
                              y1s[:, gd, :], start=(gd == 0), stop=(gd == GD - 1))
                ve.tensor_scalar(ca[:, gj, :], p[:, 1:1 + C],
                                 convw[:, gj, ti, 0:1], None, ALU.mult)
                for kk in range(1, K):
                    ve.scalar_tensor_tensor(ca[:, gj, :], p[:, 1 + kk:1 + kk + C],
                                            convw[:, gj, ti, kk:kk + 1],
                                            ca[:, gj, :], ALU.mult, ALU.add)
            for gj in range(GD):
                sc.activation(out_t[:, gj, :], ca[:, gj, :], AF.Silu,
                              bias=convb[:, gj, ti:ti + 1])

        def l2norm_inplace(src):
            """src: [128, GD, C] fp32r silu output; normalized in place."""
            sq = work.tile([128, GD, C], f16, name="sq", tag="sq", bufs=2)
            sc.activation(sq[:], src[:], AF.Square)
            ssp = _pst(ps, [1, C], "mm", 2)
            for gd in range(GD):
                te.matmul(ssp[:], ones_col, sq[:, gd, :],
                          start=(gd == 0), stop=(gd == GD - 1))
            rn = work.tile([1, C], f32, name="rn", tag="rn", bufs=2)
            sc.activation(rn[:], ssp[:], AF.Sqrt)
            ve.tensor_scalar_max(rn[:], rn[:], EPS)
            with nc.allow_low_precision("norm factor to f16"):
                rnr = work.tile([1, C], f16, name="rnr", tag="rnr", bufs=2)
                ve.reciprocal(rnr[:], rn[:])
            bc = _pst(ps, [128, C], "mm", 2)
            te.matmul(bc[:], ones_row, rnr[:])
            for gd in range(GD):
                ve.tensor_mul(src[:, gd, :], src[:, gd, :], bc[:])

        # ============ PASS 1: k/v + gradient accumulation ============
        work_cm = tc.tile_pool(name="work", bufs=1)
        work = work_cm.__enter__()
        with tc.tile_pool(name="w_kv", bufs=1) as w_kv:
            load_proj_weights(w_kv, "kv")
            for ci in range(NCH):
                xc = work.tile([128, GD, C + PAD], f16, name="xc", tag="xc",
                               bufs=2)
                sy.dma_start(xc[:], d["xT"][:, :, ci * C:ci * C + C + PAD])
                # gate statistics: sum x over this chunk's tokens
                xs_c = work.tile([128, GD, 1], f32, name="xs_c", tag="xs_c")
                ve.tensor_reduce(xs_c[:], xc[:, :, PAD:], mybir.AxisListType.X,
                                 ALU.add)
                ve.tensor_add(xsum[:], xsum[:], xs_c[:])

                k_n = work.tile([128, GD, C], f16, name="k_n", tag="k_n", bufs=2)
                proj_conv_silu("k", xc, k_n)
                vs = work.tile([128, GD, C], f16, name="vs", tag="vs", bufs=2)
                proj_conv_silu("v", xc, vs)
                l2norm_inplace(k_n)

                # forward through memory MLP (feature-major)
                a1 = work.tile([128, GO, C], f16, name="a1", tag="a1", bufs=2)
                sp1 = work.tile([128, GO, C], f16, name="sp1", tag="sp1", bufs=2)
                for hh in range(2):
                    ph = _pst(ps, [128, GD, C], "h2", 2)
                    for gg in range(GD):
                        go = hh * GD + gg
                        for gd in range(GD):
                            te.matmul(ph[:, gg, :],
                                      mw1T[:, gd, go * 128:(go + 1) * 128],
                                      k_n[:, gd, :], start=(gd == 0),
                                      stop=(gd == GD - 1))
                    sc.activation(a1[:, hh * GD:(hh + 1) * GD, :], ph[:],
                                  AF.Silu)
                    sc.activation(sp1[:, hh * GD:(hh + 1) * GD, :], ph[:],
                                  AF.Derivative_silu)
                dpredu = work.tile([128, GD, C], f16, name="dpredu",
                                   tag="dpredu", bufs=2)
                pp = _pst(ps, [128, GD, C], "h2", 2)
                for gi in range(GD):
                    for go in range(GO):
                        te.matmul(pp[:, gi, :],
                                  mw2T[:, go, gi * 128:(gi + 1) * 128],
                                  a1[:, go, :], start=(go == 0),
                                  stop=(go == GO - 1))
                ve.tensor_sub(dpredu[:], pp[:], vs[:])
                dh1 = work.tile([128, GO, C], f16, name="dh1", tag="dh1", bufs=2)
                for hh in range(2):
                    pd = _pst(ps, [128, GD, C], "h2", 2)
                    for gg in range(GD):
                        go = hh * GD + gg
                        for gi in range(GD):
                            te.matmul(pd[:, gg, :],
                                      mw2n[:, gi, go * 128:(go + 1) * 128],
                                      dpredu[:, gi, :], start=(gi == 0),
                                      stop=(gi == GD - 1))
                    ve.tensor_mul(dh1[:, hh * GD:(hh + 1) * GD, :], pd[:],
                                  sp1[:, hh * GD:(hh + 1) * GD, :])

                # transpose to token-major for the outer products
                nsub = C // 128
                k_tm = work.tile([128, nsub, DIM], f16, name="k_tm",
                                 tag="k_tm", bufs=2)
                dp_tm = work.tile([128, nsub, DIM], f16, name="dp_tm",
                                  tag="dp_tm", bufs=2)
                a1_tm = work.tile([128, nsub, HID], f16, name="a1_tm",
                                  tag="a1_tm", bufs=2)
                dh1_tm = work.tile([128, nsub, HID], f16, name="dh1_tm",
                                   tag="dh1_tm", bufs=2)
                for sub in range(nsub):
                    for src, dst, ng in ((k_n, k_tm, GD), (dpredu, dp_tm, GD),
                                         (a1, a1_tm, GO), (dh1, dh1_tm, GO)):
                        for h in range(ng // 4):
                            tp = _pst(ps, [128, 512], "sm", 2, f16)
                            for j in range(4):
                                g = h * 4 + j
                                te.transpose(tp[:, j * 128:(j + 1) * 128],
                                             src[:, g, sub * 128:(sub + 1) * 128],
                                             ident[:])
                            sc.activation(dst[:, sub, h * 512:(h + 1) * 512],
                                          tp[:], AF.Copy)
                # g1[o, d] += dh1_tm.T @ k_tm ; g2[i, o] += dp_tm.T @ a1_tm
                for go in range(GO):
                    p = _pst(ps, [128, DIM], "sm", 2)
                    for sub in range(nsub):
                        te.matmul(p[:], dh1_tm[:, sub, go * 128:(go + 1) * 128],
                                  k_tm[:, sub, :], start=(sub == 0),
                                  stop=(sub == nsub - 1))
                    ve.scalar_tensor_tensor(g1acc[:, go, :], p[:], C_LOSS,
                                            g1acc[:, go, :], ALU.mult, ALU.add)
                for gi in range(GD):
                    for h in range(2):
                        p = _pst(ps, [128, 512], "sm", 2)
                        for sub in range(nsub):
                            te.matmul(p[:],
                                      dp_tm[:, sub, gi * 128:(gi + 1) * 128],
                                      a1_tm[:, sub, h * 512:(h + 1) * 512],
                                      start=(sub == 0), stop=(sub == nsub - 1))
                        ve.scalar_tensor_tensor(g2acc[:, gi,
                                                      h * 512:(h + 1) * 512],
                                                p[:], C_LOSS,
                                                g2acc[:, gi,
                                                      h * 512:(h + 1) * 512],
                                                ALU.mult, ALU.add)

        # ---- launch collectives (overlap with pass 2) ----
        part = work.tile([128, GD, B], f32, name="part", tag="part")
        for bcol in range(B):
            ve.tensor_scalar_mul(part[:, :, bcol:bcol + 1], xsum[:],
                                 sel[:, bcol:bcol + 1])
        sy.dma_start(d["cc_x_in"][:].rearrange("p (g b) -> p g b", g=GD),
                     part[:])
        gp.collective_compute("AllReduce", ALU.add, replica_groups=REPLICA,
                              ins=[d["cc_x_in"][:]], outs=[d["cc_x_out"][:]])
        g1v = d["cc_g_in"][:, 0:SL1].rearrange("s (p d) -> p s d", p=128)
        sy.dma_start(g1v, g1acc[:])
        g2v = d["cc_g_in"][:, SL1:].rearrange(
            "(gi ph) (q o) -> ph q gi o", gi=GD, q=64)
        for ph in range(2):
            sy.dma_start(g2v[ph], g2acc[ph * 64:(ph + 1) * 64, :, :])
        gp.collective_compute("ReduceScatter", ALU.add, replica_groups=REPLICA,
                              ins=[d["cc_g_in"][:]], outs=[d["cc_g_out"][:]])
        # (work pool stays open through pass 2, closed before the tail)

        # ============ PASS 2: queries -> retrieved -> output ============
        with tc.tile_pool(name="w_q", bufs=1) as w_q:
            load_proj_weights(w_q, "q")
            woutT = w_q.tile([128, GD, DIM], f16, name="woutT")
            sy.dma_start(woutT[:], d["woutT_d"][:])
            for ci in range(NCH):
                xc = work.tile([128, GD, C + PAD], f16, name="xc", tag="xc",
                               bufs=2)
                sy.dma_start(xc[:], d["xT"][:, :, ci * C:ci * C + C + PAD])
                q_n = work.tile([128, GD, C], f16, name="q_n", tag="k_n", bufs=2)
                proj_conv_silu("q", xc, q_n)
                l2norm_inplace(q_n)
                aq = work.tile([128, GO, C], f16, name="aq", tag="a1", bufs=2)
                for hh in range(2):
                    ph = _pst(ps, [128, GD, C], "h2", 2)
                    for gg in range(GD):
                        go = hh * GD + gg
                        for gd in range(GD):
                            te.matmul(ph[:, gg, :],
                                      mw1T[:, gd, go * 128:(go + 1) * 128],
                                      q_n[:, gd, :], start=(gd == 0),
                                      stop=(gd == GD - 1))
                    sc.activation(aq[:, hh * GD:(hh + 1) * GD, :], ph[:],
                                  AF.Silu)
                r_s = work.tile([128, GD, C], f16, name="r_s", tag="dpredu", bufs=2)
                pr = _pst(ps, [128, GD, C], "h2", 2)
                for gi in range(GD):
                    for go in range(GO):
                        te.matmul(pr[:, gi, :],
                                  mw2T[:, go, gi * 128:(gi + 1) * 128],
                                  aq[:, go, :], start=(go == 0),
                                  stop=(go == GO - 1))
                sc.activation(r_s[:], pr[:], AF.Copy)
                outs = work.tile([128, GD, C], f32, name="outs", tag="outs", bufs=2)
                po = _pst(ps, [128, GD, C], "h2", 2)
                for gu in range(GD):
                    for gi in range(GD):
                        te.matmul(po[:, gu, :],
                                  woutT[:, gi, gu * 128:(gu + 1) * 128],
                                  r_s[:, gi, :], start=(gi == 0),
                                  stop=(gi == GD - 1))
                sc.activation(outs[:], po[:], AF.Copy)
                sy.dma_start(d["outT"][:, :, ci * C:(ci + 1) * C], outs[:])

        work_cm.__exit__(None, None, None)

        # ============ tail: gates, momentum/decay update ============
        with tc.tile_pool(name="tail", bufs=1) as tail:
            gwT = tail.tile([128, GD, 3, DIM], f32, name="gwT")
            sy.dma_start(gwT[:], d["gwT_d"][:])
            gateb = tail.tile([128, GD, 3], f32, name="gateb")
            sy.dma_start(gateb[:], d["gateb_d"][:])
            xsb = tail.tile([128, GD, B], f32, name="xsb")
            sy.dma_start(xsb[:],
                         d["cc_x_out"][:].rearrange("p (g b) -> p g b", g=GD))
            gsc = tail.tile([1, 3], f32, name="gsc")
            for w in range(3):
                sig = tail.tile([128, GD, B], f32, name=f"sig{w}",
                                tag="sig")
                for gj in range(GD):
                    p = _pst(ps, [128, B], "mm", 2)
                    for gd in range(GD):
                        te.matmul(p[:],
                                  gwT[:, gd, w, gj * 128:(gj + 1) * 128],
                                  xsb[:, gd, :], start=(gd == 0),
                                  stop=(gd == GD - 1))
                    sc.activation(sig[:, gj, :], p[:], AF.Sigmoid,
                                  bias=gateb[:, gj, w:w + 1], scale=1.0 / S)
                srow = tail.tile([128, 1], f32, name=f"srow{w}", tag="srow")
                ve.tensor_reduce(srow[:], sig[:], mybir.AxisListType.XY,
                                 ALU.add)
                p = _pst(ps, [1, 1], "mm", 2)
                te.matmul(p[:], ones_col32[:], srow[:])
                sc.activation(gsc[0:1, w:w + 1], p[:], AF.Copy,
                              scale=1.0 / (DIM * B))
            ones_row32 = tail.tile([1, 128], f32, name="ones_row32")
            gp.memset(ones_row32[:], 1.0)
            gbp = _pst(ps, [128, 3], "mm", 2)
            te.matmul(gbp[:], ones_row32[:], gsc[:])
            gb = tail.tile([128, 3], f32, name="gb")
            sc.activation(gb[:], gbp[:], AF.Copy)
            oneminus = tail.tile([128, 1], f32, name="oneminus")
            ve.tensor_scalar(oneminus[:], gb[:, 0:1], -1.0, 1.0, ALU.mult,
                             ALU.add)
            theta = tail.tile([128, 1], f32, name="theta")
            ve.tensor_scalar_mul(theta[:], gb[:, 1:2], MEM_LR)
            eta = tail.tile([128, 1], f32, name="eta")
            ve.tensor_scalar_mul(eta[:], gb[:, 2:3], MEM_MOM)

            g1s = tail.tile([128, DIM], f32, name="g1s")
            sy.dma_start(g1s[:],
                         d["cc_g_out"][0:SL1].rearrange("(p d) -> p d", p=128))
            g2s = tail.tile([64, HID], f32, name="g2s")
            sy.dma_start(g2s[:],
                         d["cc_g_out"][SL1:].rearrange("(q o) -> q o", q=64))
            mw1s = tail.tile([128, DIM], f32, name="mw1s")
            sy.dma_start(mw1s[:], d["mw1s_d"][:])
            mom1s = tail.tile([128, DIM], f32, name="mom1s")
            sy.dma_start(mom1s[:], d["mom1s_d"][:])
            mw2s = tail.tile([64, HID], f32, name="mw2s")
            sy.dma_start(mw2s[:], d["mw2s_d"][:])
            mom2s = tail.tile([64, HID], f32, name="mom2s")
            sy.dma_start(mom2s[:], d["mom2s_d"][:])

            for gsl, mws, moms, s_out, nw_out, np_ in (
                    (g1s, mw1s, mom1s, d["s1_s"], d["nw1_s"], 128),
                    (g2s, mw2s, mom2s, d["s2_s"], d["nw2_s"], 64)):
                t1 = tail.tile(list(gsl.shape), f32, name="t1", tag="t1")
                ve.tensor_scalar_mul(t1[:], gsl[:], theta[0:np_, :])
                st = tail.tile(list(gsl.shape), f32, name="st", tag="st")
                ve.scalar_tensor_tensor(st[:], moms[:], eta[0:np_, :], t1[:],
                                        ALU.mult, ALU.subtract)
                nwt = tail.tile(list(gsl.shape), f32, name="nwt", tag="nwt")
                ve.scalar_tensor_tensor(nwt[:], mws[:], oneminus[0:np_, :],
                                        st[:], ALU.mult, ALU.add)
                sy.dma_start(s_out[:], st[:])
                sy.dma_start(nw_out[:], nwt[:])


# ======================= host side =======================

def _gT(w, dt=np.float16):
    """(out, in=512) torch-Linear weight -> grouped lhsT [128, 4, out]."""
    return np.ascontiguousarray(
        w.T.reshape(GD, 128, -1).transpose(1, 0, 2)).astype(dt)


def _g_natural(w, groups, dt=np.float16):
    """(rows, cols) -> grouped [128, groups, cols] (rows on partitions)."""
    return np.ascontiguousarray(
        w.reshape(groups, 128, -1).transpose(1, 0, 2)).astype(dt)


def _prep(inputs):
    x = np.asarray(inputs["x"], np.float32)
    shared = {}
    for t, (a, b_) in {"k": ("w_k1", "w_k2"), "v": ("w_v1", "w_v2"),
                       "q": ("w_q1", "w_q2")}.items():
        shared[f"w1T_{t}"] = _gT(np.asarray(inputs[a], np.float32))
        shared[f"w2T_{t}"] = _gT(np.asarray(inputs[b_], np.float32))
    mem_w1 = np.asarray(inputs["mem_w1"], np.float32)
    mem_w2 = np.asarray(inputs["mem_w2"], np.float32)
    shared["mw1T"] = _gT(mem_w1)                       # [128,4,1024]
    shared["mw2T"] = _g_natural(mem_w2.T, GO)          # [128,8,512]
    shared["mw2n"] = _g_natural(mem_w2, GD)            # [128,4,1024]
    shared["woutT"] = _gT(np.asarray(inputs["w_out"], np.float32))
    gw = np.stack([_gT(np.asarray(inputs[f"gate_{t}_w"], np.float32), np.float32)
                   for t in ("d", "lr", "m")], axis=2)  # [128,4,3,512]
    shared["gwT"] = np.ascontiguousarray(gw)
    shared["ident"] = np.eye(128, dtype=np.float16)
    shared["ones"] = np.ones((128, 129), np.float16)
    cw = np.stack([np.asarray(inputs[f"conv_{t}_w"], np.float32)[:, 0, :]
                   for t in ("k", "v", "q")], axis=1)   # (512, 3, 4)
    shared["convw"] = np.ascontiguousarray(
        cw.reshape(GD, 128, 3, K).transpose(1, 0, 2, 3))
    cb = np.stack([np.asarray(inputs[f"conv_{t}_b"], np.float32)
                   for t in ("k", "v", "q")], axis=1)   # (512, 3)
    shared["convb"] = np.ascontiguousarray(
        cb.reshape(GD, 128, 3).transpose(1, 0, 2))
    gb = np.stack([np.asarray(inputs[f"gate_{t}_b"], np.float32)
                   for t in ("d", "lr", "m")], axis=1)  # (512, 3)
    shared["gateb"] = np.ascontiguousarray(
        gb.reshape(GD, 128, 3).transpose(1, 0, 2))
    mom1 = np.asarray(inputs["mom1"], np.float32)
    mom2 = np.asarray(inputs["mom2"], np.float32)

    in_maps = []
    for c in range(N_CORES):
        b = c // 2
        half = c % 2
        seq = x[b]
        if half == 0:
            seg = np.concatenate(
                [np.zeros((PAD, DIM), np.float32), seq[0:T]], axis=0)
        else:
            seg = seq[T - PAD:2 * T]
        xTg = np.ascontiguousarray(
            seg.T.reshape(GD, 128, T + PAD).transpose(1, 0, 2)).astype(np.float16)
        sel = np.zeros((128, B), np.float32)
        sel[:, b] = 1.0
        m = dict(shared)
        m["xT"] = xTg
        m["sel"] = sel
        m["mw1s"] = np.ascontiguousarray(mem_w1[c * 128:(c + 1) * 128])
        m["mom1s"] = np.ascontiguousarray(mom1[c * 128:(c + 1) * 128])
        m["mw2s"] = np.ascontiguousarray(mem_w2[c * 64:(c + 1) * 64])
        m["mom2s"] = np.ascontiguousarray(mom2[c * 64:(c + 1) * 64])
        in_maps.append(m)
    return in_maps


def _unshard(results):
    outs = []
    for c in range(N_CORES):
        a = results[c]["outT"]          # [128, 4, 4096]
        outs.append(np.moveaxis(a, 1, 0).reshape(DIM, T).T)
    output = np.concatenate(outs, axis=0).reshape(B, S, DIM)
    nw1 = np.concatenate([results[c]["nw1_s"] for c in range(N_CORES)], axis=0)
    s1 = np.concatenate([results[c]["s1_s"] for c in range(N_CORES)], axis=0)
    nw2 = np.concatenate([results[c]["nw2_s"] for c in range(N_CORES)], axis=0)
    s2 = np.concatenate([results[c]["s2_s"] for c in range(N_CORES)], axis=0)
    return (output, nw1, nw2, s1, s2)


def run(in_maps, trace=False):
    if "nc" not in _CACHE:
        _CACHE["nc"] = _build()
    return run_bass_kernel_spmd(_CACHE["nc"], in_maps,
                                core_ids=list(range(N_CORES)), trace=trace)


def kernel(**inputs):
    res = run(_prep(inputs))
    return _unshard(res.results)


if __name__ == "__main__":
    import jax
    import reference as R
    inp = {k: np.asarray(v) for k, v in R.setup_inputs().items()}
    got = kernel(**inp)
    print([g.shape for g in got])


# revision 15
# speedup vs baseline: 1.1620x; 1.0502x over previous
"""Trainium2 Bass kernel for nn_NeuralLongTermMemory.

Sharding: tokens (B*S = 32768) split 8 ways -> 4096 tokens/core (half a
batch-sequence each).  All weights replicated.  Gradients of the memory
MLP are partial-summed per core and combined with one ReduceScatter; the
data-dependent scalar gates use one small AllReduce of per-batch x sums.

Layout: feature-major ("transposed") everywhere - features on SBUF
partitions (grouped [128, G, cols]), tokens on the free dimension.  The
causal depthwise conv then becomes shifted-window ops on the free dim.
The gradient outer-products need token-major operands; those are
produced with PE transposes.

All matmuls run in float32r (full-rate fp32, ~1e-4 rel rounding).
"""

import numpy as np
import concourse.bass as bass
import concourse.mybir as mybir
import concourse.tile as tile
from concourse import bacc
from concourse.bass_utils import run_bass_kernel_spmd

f32 = mybir.dt.float32
f32r = mybir.dt.float32r
f16 = mybir.dt.float16
AF = mybir.ActivationFunctionType
ALU = mybir.AluOpType

B, S, DIM, HID, K = 4, 8192, 512, 1024, 4
N_CORES = 8
T = B * S // N_CORES        # 4096 tokens per core
HALO = K - 1                # 3
PAD = 4                     # input halo columns (even matmul free dims)
C = 256                     # chunk tokens
NCH = T // C
GD = DIM // 128             # 4
GO = HID // 128             # 8
C_LOSS = 2.0 / (B * S * DIM)
MEM_LR, MEM_MOM = 0.01, 0.9
EPS = 1e-12
SL1 = 128 * 512             # g1 slice elems per core
REPLICA = [list(range(N_CORES))]

_CACHE = {}


def _pst(ps, shape, tag, bufs, dt=f32):
    return ps.tile(shape, dt, name=tag, tag=tag, bufs=bufs, space="PSUM")


def _build():
    nc = bacc.Bacc("TRN2", target_bir_lowering=False, debug=False,
                   num_devices=N_CORES)

    def din(name, shape, dt=f16):
        return nc.dram_tensor(name, shape, dt, kind="ExternalInput")

    def dout(name, shape):
        return nc.dram_tensor(name, shape, f32, kind="ExternalOutput")

    xT = din("xT", [128, GD, T + PAD])
    w1T = {t: din(f"w1T_{t}", [128, GD, DIM]) for t in "kvq"}
    w2T = {t: din(f"w2T_{t}", [128, GD, DIM]) for t in "kvq"}
    mw1T_d = din("mw1T", [128, GD, HID])
    mw2T_d = din("mw2T", [128, GO, DIM])
    mw2n_d = din("mw2n", [128, GD, HID])
    woutT_d = din("woutT", [128, GD, DIM])
    gwT_d = din("gwT", [128, GD, 3, DIM], f32)
    ident_d = din("ident", [128, 128])
    ones_d = din("ones", [128, 129])
    convw_d = din("convw", [128, GD, 3, K], f32)
    convb_d = din("convb", [128, GD, 3], f32)
    gateb_d = din("gateb", [128, GD, 3], f32)
    sel_d = din("sel", [128, B], f32)
    mw1s_d = din("mw1s", [128, DIM], f32)
    mom1s_d = din("mom1s", [128, DIM], f32)
    mw2s_d = din("mw2s", [64, HID], f32)
    mom2s_d = din("mom2s", [64, HID], f32)

    outT = dout("outT", [128, GD, T])
    nw1_s = dout("nw1_s", [128, DIM])
    s1_s = dout("s1_s", [128, DIM])
    nw2_s = dout("nw2_s", [64, HID])
    s2_s = dout("s2_s", [64, HID])

    # collective scratch
    cc_g_in = nc.dram_tensor("cc_g_in", [N_CORES, 2 * SL1], f32)
    cc_g_out = nc.dram_tensor("cc_g_out", [2 * SL1], f32)
    cc_x_in = nc.dram_tensor("cc_x_in", [128, GD * B], f32)
    cc_x_out = nc.dram_tensor("cc_x_out", [128, GD * B], f32,
                              addr_space="Shared")

    with tile.TileContext(nc) as tc:
        _emit(nc, tc, locals())
    nc.compile()
    return nc


def _emit(nc, tc, d):
    sc, ve, te, sy, gp = nc.scalar, nc.vector, nc.tensor, nc.sync, nc.gpsimd

    with (
        tc.tile_pool(name="consts", bufs=1) as consts,
        tc.tile_pool(name="accs", bufs=1) as accs,
        tc.tile_pool(name="wmem", bufs=1) as wmem,
        tc.tile_pool(name="ps", bufs=1, space="PSUM") as ps,
    ):
        # ---- constants / resident weights ----
        ident = consts.tile([128, 128], f16)
        sy.dma_start(ident[:], d["ident_d"][:])
        onesb = consts.tile([128, 129], f16)
        sy.dma_start(onesb[:], d["ones_d"][:])
        ones_col = onesb[:, 0:1]
        ones_row = onesb[0:1, 1:129]
        ones_col32 = consts.tile([128, 1], f32)
        gp.memset(ones_col32[:], 1.0)
        convw = consts.tile([128, GD, 3, K], f32)
        sy.dma_start(convw[:], d["convw_d"][:])
        convb = consts.tile([128, GD, 3], f32)
        sy.dma_start(convb[:], d["convb_d"][:])
        sel = consts.tile([128, B], f32)
        sy.dma_start(sel[:], d["sel_d"][:])

        mw1T = wmem.tile([128, GD, HID], f16)
        sy.dma_start(mw1T[:], d["mw1T_d"][:])
        mw2T = wmem.tile([128, GO, DIM], f16)
        sy.dma_start(mw2T[:], d["mw2T_d"][:])
        mw2n = wmem.tile([128, GD, HID], f16)
        sy.dma_start(mw2n[:], d["mw2n_d"][:])

        g1acc = accs.tile([128, GO, DIM], f32)
        gp.memset(g1acc[:], 0.0)
        g2acc = accs.tile([128, GD, HID], f32)
        gp.memset(g2acc[:], 0.0)
        xsum = accs.tile([128, GD, 1], f32)
        gp.memset(xsum[:], 0.0)

        w1sb = {}
        w2sb = {}

        def load_proj_weights(pool, tensors):
            for t in tensors:
                wa = pool.tile([128, GD, DIM], f16, name=f"w1sb_{t}",
                               tag=f"w1sb_{t}")
                sy.dma_start(wa[:], d["w1T"][t][:])
                w1sb[t] = wa
                wb = pool.tile([128, GD, DIM], f16, name=f"w2sb_{t}",
                               tag=f"w2sb_{t}")
                sy.dma_start(wb[:], d["w2T"][t][:])
                w2sb[t] = wb

        def proj_conv_silu(tn, xc, out_t):
            ncols = C + PAD
            y1s = work.tile([128, GD, ncols], f16, name="y1s", tag="y1s",
                            bufs=2)
            for gj in range(GD):
                p = _pst(ps, [128, ncols], "mm", 2)
                for gd in range(GD):
                    te.matmul(p[:], w1sb[tn][:, gd, gj * 128:(gj + 1) * 128],
                              xc[:, gd, :], start=(gd == 0), stop=(gd == GD - 1))
                sc.activation(y1s[:, gj, :], p[:], AF.Silu)
            ca = work.tile([128, GD, C], f32, name="ca", tag="ca", bufs=2)
            ti = "kvq".index(tn)
            for gj in range(GD):
                p = _pst(ps, [128, ncols], "mm", 2)
                for gd in range(GD):
                    te.matmul(p[:], w2sb[tn][:, gd, gj * 128:(gj + 1) * 128],
                              y1s[:, gd, :], start=(gd == 0), stop=(gd == GD - 1))
                ve.tensor_scalar(ca[:, gj, :], p[:, 1:1 + C],
                                 convw[:, gj, ti, 0:1], None, ALU.mult)
                for kk in range(1, K):
                    ve.scalar_tensor_tensor(ca[:, gj, :], p[:, 1 + kk:1 + kk + C],
                                            convw[:, gj, ti, kk:kk + 1],
                                            ca[:, gj, :], ALU.mult, ALU.add)
            for gj in range(GD):
                sc.activation(out_t[:, gj, :], ca[:, gj, :], AF.Silu,
                              bias=convb[:, gj, ti:ti + 1])

        def l2norm_inplace(src):
            """src: [128, GD, C] fp32r silu output; normalized in place."""
            sq = work.tile([128, GD, C], f16, name="sq", tag="sq", bufs=2)
            sc.activation(sq[:], src[:], AF.Square)
            ssp = _pst(ps, [1, C], "mm", 2)
            for gd in range(GD):
                te.matmul(ssp[:], ones_col, sq[:, gd, :],
                          start=(gd == 0), stop=(gd == GD - 1))
            rn = work.tile([1, C], f32, name="rn", tag="rn", bufs=2)
            sc.activation(rn[:], ssp[:], AF.Sqrt)
            ve.tensor_scalar_max(rn[:], rn[:], EPS)
            with nc.allow_low_precision("norm factor to f16"):
                rnr = work.tile([1, C], f16, name="rnr", tag="rnr", bufs=2)
                ve.reciprocal(rnr[:], rn[:])
            bc = _pst(ps, [128, C], "mm", 2)
            te.matmul(bc[:], ones_row, rnr[:])
            for gd in range(GD):
                ve.tensor_mul(src[:, gd, :], src[:, gd, :], bc[:])

        # ============ PASS 1: k/v + gradient accumulation ============
        work_cm = tc.tile_pool(name="work", bufs=1)
        work = work_cm.__enter__()
        with tc.tile_pool(name="w_kv", bufs=1) as w_kv:
            load_proj_weights(w_kv, "kv")
            def p1_stage_a(ci):
                xc = work.tile([128, GD, C + PAD], f16, name="xc", tag="xc",
                               bufs=3)
                sy.dma_start(xc[:], d["xT"][:, :, ci * C:ci * C + C + PAD])
                xs_c = work.tile([128, GD, 1], f32, name="xs_c", tag="xs_c")
                ve.tensor_reduce(xs_c[:], xc[:, :, PAD:], mybir.AxisListType.X,
                                 ALU.add)
                ve.tensor_add(xsum[:], xsum[:], xs_c[:])
                k_n = work.tile([128, GD, C], f16, name="k_n", tag="k_n",
                                bufs=2)
                proj_conv_silu("k", xc, k_n)
                vs = work.tile([128, GD, C], f16, name="vs", tag="vs", bufs=2)
                proj_conv_silu("v", xc, vs)
                return k_n, vs

            def p1_stage_b(st):
                k_n, vs = st
                l2norm_inplace(k_n)
                a1 = work.tile([128, GO, C], f16, name="a1", tag="a1", bufs=2)
                sp1 = work.tile([128, GO, C], f16, name="sp1", tag="sp1",
                                bufs=2)
                phs = []
                for hh in range(2):
                    ph = _pst(ps, [128, GD, C], "h2", 2)
                    for gg in range(GD):
                        go = hh * GD + gg
                        for gd in range(GD):
                            te.matmul(ph[:, gg, :],
                                      mw1T[:, gd, go * 128:(go + 1) * 128],
                                      k_n[:, gd, :], start=(gd == 0),
                                      stop=(gd == GD - 1))
                    phs.append(ph)
                for hh in range(2):
                    sc.activation(a1[:, hh * GD:(hh + 1) * GD, :], phs[hh][:],
                                  AF.Silu)
                for hh in range(2):
                    sc.activation(sp1[:, hh * GD:(hh + 1) * GD, :], phs[hh][:],
                                  AF.Derivative_silu)
                dpredu = work.tile([128, GD, C], f16, name="dpredu",
                                   tag="dpredu", bufs=2)
                pp = _pst(ps, [128, GD, C], "h2", 2)
                for gi in range(GD):
                    for go in range(GO):
                        te.matmul(pp[:, gi, :],
                                  mw2T[:, go, gi * 128:(gi + 1) * 128],
                                  a1[:, go, :], start=(go == 0),
                                  stop=(go == GO - 1))
                ve.tensor_sub(dpredu[:], pp[:], vs[:])
                dh1 = work.tile([128, GO, C], f16, name="dh1", tag="dh1",
                                bufs=2)
                for hh in range(2):
                    pd = _pst(ps, [128, GD, C], "h2", 2)
                    for gg in range(GD):
                        go = hh * GD + gg
                        for gi in range(GD):
                            te.matmul(pd[:, gg, :],
                                      mw2n[:, gi, go * 128:(go + 1) * 128],
                                      dpredu[:, gi, :], start=(gi == 0),
                                      stop=(gi == GD - 1))
                    ve.tensor_mul(dh1[:, hh * GD:(hh + 1) * GD, :], pd[:],
                                  sp1[:, hh * GD:(hh + 1) * GD, :])
                nsub = C // 128
                k_tm = work.tile([128, nsub, DIM], f16, name="k_tm",
                                 tag="k_tm", bufs=2)
                dp_tm = work.tile([128, nsub, DIM], f16, name="dp_tm",
                                  tag="dp_tm", bufs=2)
                a1_tm = work.tile([128, nsub, HID], f16, name="a1_tm",
                                  tag="a1_tm", bufs=2)
                dh1_tm = work.tile([128, nsub, HID], f16, name="dh1_tm",
                                   tag="dh1_tm", bufs=2)
                for sub in range(nsub):
                    for src, dst, ng in ((k_n, k_tm, GD), (dpredu, dp_tm, GD),
                                         (a1, a1_tm, GO), (dh1, dh1_tm, GO)):
                        for h in range(ng // 4):
                            tp = _pst(ps, [128, 512], "sm", 2, f16)
                            for j in range(4):
                                g = h * 4 + j
                                te.transpose(tp[:, j * 128:(j + 1) * 128],
                                             src[:, g, sub * 128:(sub + 1) * 128],
                                             ident[:])
                            sc.activation(dst[:, sub, h * 512:(h + 1) * 512],
                                          tp[:], AF.Copy)
                for go in range(GO):
                    p = _pst(ps, [128, DIM], "sm", 2)
                    for sub in range(nsub):
                        te.matmul(p[:], dh1_tm[:, sub, go * 128:(go + 1) * 128],
                                  k_tm[:, sub, :], start=(sub == 0),
                                  stop=(sub == nsub - 1))
                    ve.scalar_tensor_tensor(g1acc[:, go, :], p[:], C_LOSS,
                                            g1acc[:, go, :], ALU.mult, ALU.add)
                for gi in range(GD):
                    for h in range(2):
                        p = _pst(ps, [128, 512], "sm", 2)
                        for sub in range(nsub):
                            te.matmul(p[:],
                                      dp_tm[:, sub, gi * 128:(gi + 1) * 128],
                                      a1_tm[:, sub, h * 512:(h + 1) * 512],
                                      start=(sub == 0), stop=(sub == nsub - 1))
                        ve.scalar_tensor_tensor(g2acc[:, gi,
                                                      h * 512:(h + 1) * 512],
                                                p[:], C_LOSS,
                                                g2acc[:, gi,
                                                      h * 512:(h + 1) * 512],
                                                ALU.mult, ALU.add)

            prev1 = None
            for ci in range(NCH):
                cur1 = p1_stage_a(ci)
                if prev1 is not None:
                    p1_stage_b(prev1)
                prev1 = cur1
            p1_stage_b(prev1)

        # ---- launch collectives (overlap with pass 2) ----
        part = work.tile([128, GD, B], f32, name="part", tag="part")
        for bcol in range(B):
            ve.tensor_scalar_mul(part[:, :, bcol:bcol + 1], xsum[:],
                                 sel[:, bcol:bcol + 1])
        sy.dma_start(d["cc_x_in"][:].rearrange("p (g b) -> p g b", g=GD),
                     part[:])
        gp.collective_compute("AllReduce", ALU.add, replica_groups=REPLICA,
                              ins=[d["cc_x_in"][:]], outs=[d["cc_x_out"][:]])
        g1v = d["cc_g_in"][:, 0:SL1].rearrange("s (p d) -> p s d", p=128)
        sy.dma_start(g1v, g1acc[:])
        g2v = d["cc_g_in"][:, SL1:].rearrange(
            "(gi ph) (q o) -> ph q gi o", gi=GD, q=64)
        for ph in range(2):
            sy.dma_start(g2v[ph], g2acc[ph * 64:(ph + 1) * 64, :, :])
        gp.collective_compute("ReduceScatter", ALU.add, replica_groups=REPLICA,
                              ins=[d["cc_g_in"][:]], outs=[d["cc_g_out"][:]])
        # (work pool stays open through pass 2, closed before the tail)

        # ============ PASS 2: queries -> retrieved -> output ============
        with tc.tile_pool(name="w_q", bufs=1) as w_q:
            load_proj_weights(w_q, "q")
            woutT = w_q.tile([128, GD, DIM], f16, name="woutT")
            sy.dma_start(woutT[:], d["woutT_d"][:])
            def p2_stage_a(ci):
                xc = work.tile([128, GD, C + PAD], f16, name="xc", tag="xc",
                               bufs=3)
                sy.dma_start(xc[:], d["xT"][:, :, ci * C:ci * C + C + PAD])
                q_n = work.tile([128, GD, C], f16, name="q_n", tag="k_n",
                                bufs=2)
                proj_conv_silu("q", xc, q_n)
                return ci, q_n

            def p2_stage_b(st):
                ci, q_n = st
                l2norm_inplace(q_n)
                aq = work.tile([128, GO, C], f16, name="aq", tag="a1", bufs=2)
                phs = []
                for hh in range(2):
                    ph = _pst(ps, [128, GD, C], "h2", 2)
                    for gg in range(GD):
                        go = hh * GD + gg
                        for gd in range(GD):
                            te.matmul(ph[:, gg, :],
                                      mw1T[:, gd, go * 128:(go + 1) * 128],
                                      q_n[:, gd, :], start=(gd == 0),
                                      stop=(gd == GD - 1))
                    phs.append(ph)
                for hh in range(2):
                    sc.activation(aq[:, hh * GD:(hh + 1) * GD, :], phs[hh][:],
                                  AF.Silu)
                r_s = work.tile([128, GD, C], f16, name="r_s", tag="dpredu",
                                bufs=2)
                pr = _pst(ps, [128, GD, C], "h2", 2)
                for gi in range(GD):
                    for go in range(GO):
                        te.matmul(pr[:, gi, :],
                                  mw2T[:, go, gi * 128:(gi + 1) * 128],
                                  aq[:, go, :], start=(go == 0),
                                  stop=(go == GO - 1))
                sc.activation(r_s[:], pr[:], AF.Copy)
                outs = work.tile([128, GD, C], f32, name="outs", tag="outs",
                                 bufs=2)
                po = _pst(ps, [128, GD, C], "h2", 2)
                for gu in range(GD):
                    for gi in range(GD):
                        te.matmul(po[:, gu, :],
                                  woutT[:, gi, gu * 128:(gu + 1) * 128],
                                  r_s[:, gi, :], start=(gi == 0),
                                  stop=(gi == GD - 1))
                sc.activation(outs[:], po[:], AF.Copy)
                sy.dma_start(d["outT"][:, :, ci * C:(ci + 1) * C], outs[:])

            prev2 = None
            for ci in range(NCH):
                cur2 = p2_stage_a(ci)
                if prev2 is not None:
                    p2_stage_b(prev2)
                prev2 = cur2
            p2_stage_b(prev2)

        work_cm.__exit__(None, None, None)

        # ============ tail: gates, momentum/decay update ============
        with tc.tile_pool(name="tail", bufs=1) as tail:
            gwT = tail.tile([128, GD, 3, DIM], f32, name="gwT")
            sy.dma_start(gwT[:], d["gwT_d"][:])
            gateb = tail.tile([128, GD, 3], f32, name="gateb")
            sy.dma_start(gateb[:], d["gateb_d"][:])
            xsb = tail.tile([128, GD, B], f32, name="xsb")
            sy.dma_start(xsb[:],
                         d["cc_x_out"][:].rearrange("p (g b) -> p g b", g=GD))
            gsc = tail.tile([1, 3], f32, name="gsc")
            for w in range(3):
                sig = tail.tile([128, GD, B], f32, name=f"sig{w}",
                                tag="sig")
                for gj in range(GD):
                    p = _pst(ps, [128, B], "mm", 2)
                    for gd in range(GD):
                        te.matmul(p[:],
                                  gwT[:, gd, w, gj * 128:(gj + 1) * 128],
                                  xsb[:, gd, :], start=(gd == 0),
                                  stop=(gd == GD - 1))
                    sc.activation(sig[:, gj, :], p[:], AF.Sigmoid,
                                  bias=gateb[:, gj, w:w + 1], scale=1.0 / S)
                srow = tail.tile([128, 1], f32, name=f"srow{w}", tag="srow")
                ve.tensor_reduce(srow[:], sig[:], mybir.AxisListType.XY,
                                 ALU.add)
                p = _pst(ps, [1, 1], "mm", 2)
                te.matmul(p[:], ones_col32[:], srow[:])
                sc.activation(gsc[0:1, w:w + 1], p[:], AF.Copy,
                              scale=1.0 / (DIM * B))
            ones_row32 = tail.tile([1, 128], f32, name="ones_row32")
            gp.memset(ones_row32[:], 1.0)
            gbp = _pst(ps, [128, 3], "mm", 2)
            te.matmul(gbp[:], ones_row32[:], gsc[:])
            gb = tail.tile([128, 3], f32, name="gb")
            sc.activation(gb[:], gbp[:], AF.Copy)
            oneminus = tail.tile([128, 1], f32, name="oneminus")
            ve.tensor_scalar(oneminus[:], gb[:, 0:1], -1.0, 1.0, ALU.mult,
                             ALU.add)
            theta = tail.tile([128, 1], f32, name="theta")
            ve.tensor_scalar_mul(theta[:], gb[:, 1:2], MEM_LR)
            eta = tail.tile([128, 1], f32, name="eta")
            ve.tensor_scalar_mul(eta[:], gb[:, 2:3], MEM_MOM)

            g1s = tail.tile([128, DIM], f32, name="g1s")
            sy.dma_start(g1s[:],
                         d["cc_g_out"][0:SL1].rearrange("(p d) -> p d", p=128))
            g2s = tail.tile([64, HID], f32, name="g2s")
            sy.dma_start(g2s[:],
                         d["cc_g_out"][SL1:].rearrange("(q o) -> q o", q=64))
            mw1s = tail.tile([128, DIM], f32, name="mw1s")
            sy.dma_start(mw1s[:], d["mw1s_d"][:])
            mom1s = tail.tile([128, DIM], f32, name="mom1s")
            sy.dma_start(mom1s[:], d["mom1s_d"][:])
            mw2s = tail.tile([64, HID], f32, name="mw2s")
            sy.dma_start(mw2s[:], d["mw2s_d"][:])
            mom2s = tail.tile([64, HID], f32, name="mom2s")
            sy.dma_start(mom2s[:], d["mom2s_d"][:])

            for gsl, mws, moms, s_out, nw_out, np_ in (
                    (g1s, mw1s, mom1s, d["s1_s"], d["nw1_s"], 128),
                    (g2s, mw2s, mom2s, d["s2_s"], d["nw2_s"], 64)):
                t1 = tail.tile(list(gsl.shape), f32, name="t1", tag="t1")
                ve.tensor_scalar_mul(t1[:], gsl[:], theta[0:np_, :])
                st = tail.tile(list(gsl.shape), f32, name="st", tag="st")
                ve.scalar_tensor_tensor(st[:], moms[:], eta[0:np_, :], t1[:],
                                        ALU.mult, ALU.subtract)
                nwt = tail.tile(list(gsl.shape), f32, name="nwt", tag="nwt")
                ve.scalar_tensor_tensor(nwt[:], mws[:], oneminus[0:np_, :],
                                        st[:], ALU.mult, ALU.add)
                sy.dma_start(s_out[:], st[:])
                sy.dma_start(nw_out[:], nwt[:])


# ======================= host side =======================

def _gT(w, dt=np.float16):
    """(out, in=512) torch-Linear weight -> grouped lhsT [128, 4, out]."""
    return np.ascontiguousarray(
        w.T.reshape(GD, 128, -1).transpose(1, 0, 2)).astype(dt)


def _g_natural(w, groups, dt=np.float16):
    """(rows, cols) -> grouped [128, groups, cols] (rows on partitions)."""
    return np.ascontiguousarray(
        w.reshape(groups, 128, -1).transpose(1, 0, 2)).astype(dt)


def _prep(inputs):
    x = np.asarray(inputs["x"], np.float32)
    shared = {}
    for t, (a, b_) in {"k": ("w_k1", "w_k2"), "v": ("w_v1", "w_v2"),
                       "q": ("w_q1", "w_q2")}.items():
        shared[f"w1T_{t}"] = _gT(np.asarray(inputs[a], np.float32))
        shared[f"w2T_{t}"] = _gT(np.asarray(inputs[b_], np.float32))
    mem_w1 = np.asarray(inputs["mem_w1"], np.float32)
    mem_w2 = np.asarray(inputs["mem_w2"], np.float32)
    shared["mw1T"] = _gT(mem_w1)                       # [128,4,1024]
    shared["mw2T"] = _g_natural(mem_w2.T, GO)          # [128,8,512]
    shared["mw2n"] = _g_natural(mem_w2, GD)            # [128,4,1024]
    shared["woutT"] = _gT(np.asarray(inputs["w_out"], np.float32))
    gw = np.stack([_gT(np.asarray(inputs[f"gate_{t}_w"], np.float32), np.float32)
                   for t in ("d", "lr", "m")], axis=2)  # [128,4,3,512]
    shared["gwT"] = np.ascontiguousarray(gw)
    shared["ident"] = np.eye(128, dtype=np.float16)
    shared["ones"] = np.ones((128, 129), np.float16)
    cw = np.stack([np.asarray(inputs[f"conv_{t}_w"], np.float32)[:, 0, :]
                   for t in ("k", "v", "q")], axis=1)   # (512, 3, 4)
    shared["convw"] = np.ascontiguousarray(
        cw.reshape(GD, 128, 3, K).transpose(1, 0, 2, 3))
    cb = np.stack([np.asarray(inputs[f"conv_{t}_b"], np.float32)
                   for t in ("k", "v", "q")], axis=1)   # (512, 3)
    shared["convb"] = np.ascontiguousarray(
        cb.reshape(GD, 128, 3).transpose(1, 0, 2))
    gb = np.stack([np.asarray(inputs[f"gate_{t}_b"], np.float32)
                   for t in ("d", "lr", "m")], axis=1)  # (512, 3)
    shared["gateb"] = np.ascontiguousarray(
        gb.reshape(GD, 128, 3).transpose(1, 0, 2))
    mom1 = np.asarray(inputs["mom1"], np.float32)
    mom2 = np.asarray(inputs["mom2"], np.float32)

    in_maps = []
    for c in range(N_CORES):
        b = c // 2
        half = c % 2
        seq = x[b]
        if half == 0:
            seg = np.concatenate(
                [np.zeros((PAD, DIM), np.float32), seq[0:T]], axis=0)
        else:
            seg = seq[T - PAD:2 * T]
        xTg = np.ascontiguousarray(
            seg.T.reshape(GD, 128, T + PAD).transpose(1, 0, 2)).astype(np.float16)
        sel = np.zeros((128, B), np.float32)
        sel[:, b] = 1.0
        m = dict(shared)
        m["xT"] = xTg
        m["sel"] = sel
        m["mw1s"] = np.ascontiguousarray(mem_w1[c * 128:(c + 1) * 128])
        m["mom1s"] = np.ascontiguousarray(mom1[c * 128:(c + 1) * 128])
        m["mw2s"] = np.ascontiguousarray(mem_w2[c * 64:(c + 1) * 64])
        m["mom2s"] = np.ascontiguousarray(mom2[c * 64:(c + 1) * 64])
        in_maps.append(m)
    return in_maps


def _unshard(results):
    outs = []
    for c in range(N_CORES):
        a = results[c]["outT"]          # [128, 4, 4096]
        outs.append(np.moveaxis(a, 1, 0).reshape(DIM, T).T)
    output = np.concatenate(outs, axis=0).reshape(B, S, DIM)
    nw1 = np.concatenate([results[c]["nw1_s"] for c in range(N_CORES)], axis=0)
    s1 = np.concatenate([results[c]["s1_s"] for c in range(N_CORES)], axis=0)
    nw2 = np.concatenate([results[c]["nw2_s"] for c in range(N_CORES)], axis=0)
    s2 = np.concatenate([results[c]["s2_s"] for c in range(N_CORES)], axis=0)
    return (output, nw1, nw2, s1, s2)


def run(in_maps, trace=False):
    if "nc" not in _CACHE:
        _CACHE["nc"] = _build()
    return run_bass_kernel_spmd(_CACHE["nc"], in_maps,
                                core_ids=list(range(N_CORES)), trace=trace)


def kernel(**inputs):
    res = run(_prep(inputs))
    return _unshard(res.results)


if __name__ == "__main__":
    import jax
    import reference as R
    inp = {k: np.asarray(v) for k, v in R.setup_inputs().items()}
    got = kernel(**inp)
    print([g.shape for g in got])


# revision 20
# speedup vs baseline: 1.5545x; 1.3377x over previous
"""Trainium2 Bass kernel for nn_NeuralLongTermMemory.

Sharding: tokens (B*S = 32768) split 8 ways -> 4096 tokens/core (half a
batch-sequence each).  All weights replicated.  Gradients of the memory
MLP are partial-summed per core and combined with one ReduceScatter; the
data-dependent scalar gates use one small AllReduce of per-batch x sums.

Layout: feature-major - features on SBUF partitions (grouped
[128, G, cols]), tokens on the free dimension.  The causal depthwise
conv is shifted-window multiply-adds on the free dim with a 3-column
carry between chunks.  Gradient outer-products need token-major
operands, produced with PE transposes.

Matmul operands are fp16 (fp32 PSUM accumulation); gradient
accumulators, gates, and outputs stay fp32.
"""

import numpy as np
import concourse.bass as bass
import concourse.mybir as mybir
import concourse.tile as tile
from concourse import bacc
from concourse.bass_utils import run_bass_kernel_spmd

f32 = mybir.dt.float32
f16 = mybir.dt.float16
AF = mybir.ActivationFunctionType
ALU = mybir.AluOpType

B, S, DIM, HID, K = 4, 8192, 512, 1024, 4
N_CORES = 8
T = B * S // N_CORES        # 4096 tokens per core
PAD = 4                     # x halo columns at the sequence head
C = 512                     # chunk tokens
NCH = T // C
NSUB = C // 128
GD = DIM // 128             # 4
GO = HID // 128             # 8
C_LOSS = 2.0 / (B * S * DIM)
MEM_LR, MEM_MOM = 0.01, 0.9
EPS = 1e-12
SL1 = 128 * 512             # g1 slice elems per core
REPLICA = [list(range(N_CORES))]

_CACHE = {}


def _pst(ps, shape, tag, bufs, dt=f32):
    return ps.tile(shape, dt, name=tag, tag=tag, bufs=bufs, space="PSUM")


def _build():
    nc = bacc.Bacc("TRN2", target_bir_lowering=False, debug=False,
                   num_devices=N_CORES)

    def din(name, shape, dt=f16):
        return nc.dram_tensor(name, shape, dt, kind="ExternalInput")

    def dout(name, shape):
        return nc.dram_tensor(name, shape, f32, kind="ExternalOutput")

    xT = din("xT", [128, GD, T + PAD])
    w1T = {t: din(f"w1T_{t}", [128, GD, DIM]) for t in "kvq"}
    w2T = {t: din(f"w2T_{t}", [128, GD, DIM]) for t in "kvq"}
    mw1T_d = din("mw1T", [128, GD, HID])
    mw2T_d = din("mw2T", [128, GO, DIM])
    mw2n_d = din("mw2n", [128, GD, HID])
    woutT_d = din("woutT", [128, GD, DIM])
    gwT_d = din("gwT", [128, GD, 3, DIM], f32)
    ident_d = din("ident", [128, 128])
    ones_d = din("ones", [128, 129])
    convw_d = din("convw", [128, GD, 3, K], f32)
    convb_d = din("convb", [128, GD, 3], f32)
    gateb_d = din("gateb", [128, GD, 3], f32)
    sel_d = din("sel", [128, B], f32)
    mw1s_d = din("mw1s", [128, DIM], f32)
    mom1s_d = din("mom1s", [128, DIM], f32)
    mw2s_d = din("mw2s", [64, HID], f32)
    mom2s_d = din("mom2s", [64, HID], f32)

    outT = dout("outT", [128, GD, T])
    nw1_s = dout("nw1_s", [128, DIM])
    s1_s = dout("s1_s", [128, DIM])
    nw2_s = dout("nw2_s", [64, HID])
    s2_s = dout("s2_s", [64, HID])

    cc_g_in = nc.dram_tensor("cc_g_in", [N_CORES, 2 * SL1], f32)
    cc_g_out = nc.dram_tensor("cc_g_out", [2 * SL1], f32)
    cc_x_in = nc.dram_tensor("cc_x_in", [128, GD * B], f32)
    cc_x_out = nc.dram_tensor("cc_x_out", [128, GD * B], f32,
                              addr_space="Shared")

    with tile.TileContext(nc) as tc:
        _emit(nc, tc, locals())
    nc.compile()
    return nc


def _emit(nc, tc, d):
    sc, ve, te, sy, gp = nc.scalar, nc.vector, nc.tensor, nc.sync, nc.gpsimd

    with (
        tc.tile_pool(name="consts", bufs=1) as consts,
        tc.tile_pool(name="accs", bufs=1) as accs,
        tc.tile_pool(name="wmem", bufs=1) as wmem,
        tc.tile_pool(name="ps", bufs=1, space="PSUM") as ps,
    ):
        # ---- constants / resident weights ----
        ident = consts.tile([128, 128], f16)
        sy.dma_start(ident[:], d["ident_d"][:])
        onesb = consts.tile([128, 129], f16)
        sy.dma_start(onesb[:], d["ones_d"][:])
        ones_col = onesb[:, 0:1]
        ones_row = onesb[0:1, 1:129]
        ones_col32 = consts.tile([128, 1], f32)
        gp.memset(ones_col32[:], 1.0)
        convw = consts.tile([128, GD, 3, K], f32)
        sy.dma_start(convw[:], d["convw_d"][:])
        convb = consts.tile([128, GD, 3], f32)
        sy.dma_start(convb[:], d["convb_d"][:])
        sel = consts.tile([128, B], f32)
        sy.dma_start(sel[:], d["sel_d"][:])

        mw1T = wmem.tile([128, GD, HID], f16)
        sy.dma_start(mw1T[:], d["mw1T_d"][:])
        mw2T = wmem.tile([128, GO, DIM], f16)
        sy.dma_start(mw2T[:], d["mw2T_d"][:])
        mw2n = wmem.tile([128, GD, HID], f16)
        sy.dma_start(mw2n[:], d["mw2n_d"][:])

        g1acc = accs.tile([128, GO, DIM], f32)
        gp.memset(g1acc[:], 0.0)
        g2acc = accs.tile([128, GD, HID], f32)
        gp.memset(g2acc[:], 0.0)
        xsum = accs.tile([128, GD, 1], f32)
        gp.memset(xsum[:], 0.0)

        w1sb = {}
        w2sb = {}

        def load_proj_weights(pool, tensors):
            for t in tensors:
                wa = pool.tile([128, GD, DIM], f16, name=f"w1sb_{t}",
                               tag=f"w1sb_{t}")
                sy.dma_start(wa[:], d["w1T"][t][:])
                w1sb[t] = wa
                wb = pool.tile([128, GD, DIM], f16, name=f"w2sb_{t}",
                               tag=f"w2sb_{t}")
                sy.dma_start(wb[:], d["w2T"][t][:])
                w2sb[t] = wb

        def proj_halo(tn):
            """proj2(silu(proj1(x))) for the 4 head-halo tokens."""
            xch = work.tile([128, GD, PAD], f16, name="xch", tag="xch")
            sy.dma_start(xch[:], d["xT"][:, :, 0:PAD])
            y1hs = work.tile([128, GD, PAD], f16, name="y1hs", tag="y1hs")
            for gj in range(GD):
                p = _pst(ps, [128, PAD], "mm", 2)
                for gd in range(GD):
                    te.matmul(p[:], w1sb[tn][:, gd, gj * 128:(gj + 1) * 128],
                              xch[:, gd, :], start=(gd == 0),
                              stop=(gd == GD - 1))
                sc.activation(y1hs[:, gj, :], p[:], AF.Silu)
            yh = work.tile([128, GD, PAD], f16, name=f"yh{tn}", tag=f"yh_{tn}")
            for gj in range(GD):
                p = _pst(ps, [128, PAD], "mm", 2)
                for gd in range(GD):
                    te.matmul(p[:], w2sb[tn][:, gd, gj * 128:(gj + 1) * 128],
                              y1hs[:, gd, :], start=(gd == 0),
                              stop=(gd == GD - 1))
                sc.activation(yh[:, gj, :], p[:], AF.Copy)
            return yh

        def proj_conv_silu(tn, xc, out_t, carry):
            """proj2(silu(proj1(x))) -> y2 (sbuf, 3-col carry prefix) ->
            causal conv -> +bias -> silu -> out_t.  Returns y2 (next
            chunk's carry source)."""
            ti = "kvq".index(tn)
            y1s = work.tile([128, GD, C], f16, name="y1s", tag="y1s")
            for gj in range(GD):
                p = _pst(ps, [128, C], "mm", 2)
                for gd in range(GD):
                    te.matmul(p[:], w1sb[tn][:, gd, gj * 128:(gj + 1) * 128],
                              xc[:, gd, :], start=(gd == 0),
                              stop=(gd == GD - 1))
                sc.activation(y1s[:, gj, :], p[:], AF.Silu)
            y2tag = "y2_a" if tn in "kq" else "y2_b"
            y2 = work.tile([128, GD, C + 3], f16, name=f"y2{tn}",
                           tag=y2tag, bufs=2)
            ve.tensor_copy(y2[:, :, 0:3], carry)
            for gj in range(GD):
                p = _pst(ps, [128, C], "mm", 2)
                for gd in range(GD):
                    te.matmul(p[:], w2sb[tn][:, gd, gj * 128:(gj + 1) * 128],
                              y1s[:, gd, :], start=(gd == 0),
                              stop=(gd == GD - 1))
                sc.activation(y2[:, gj, 3:C + 3], p[:], AF.Copy)
            ca = work.tile([128, GD, C], f16, name="ca", tag="ca")
            for gj in range(GD):
                ve.tensor_scalar(ca[:, gj, :], y2[:, gj, 0:C],
                                 convw[:, gj, ti, 0:1], None, ALU.mult)
                for kk in range(1, K):
                    ve.scalar_tensor_tensor(ca[:, gj, :], y2[:, gj, kk:kk + C],
                                            convw[:, gj, ti, kk:kk + 1],
                                            ca[:, gj, :], ALU.mult, ALU.add)
            for gj in range(GD):
                sc.activation(out_t[:, gj, :], ca[:, gj, :], AF.Silu,
                              bias=convb[:, gj, ti:ti + 1])
            return y2

        def l2norm_inplace(src):
            sq = work.tile([128, GD, C], f16, name="sq", tag="sq")
            sc.activation(sq[:], src[:], AF.Square)
            ssp = _pst(ps, [1, C], "mm", 2)
            for gd in range(GD):
                te.matmul(ssp[:], ones_col, sq[:, gd, :],
                          start=(gd == 0), stop=(gd == GD - 1))
            rn = work.tile([1, C], f32, name="rn", tag="rn", bufs=2)
            sc.activation(rn[:], ssp[:], AF.Sqrt)
            ve.tensor_scalar_max(rn[:], rn[:], EPS)
            with nc.allow_low_precision("norm factor to f16"):
                rnr = work.tile([1, C], f16, name="rnr", tag="rnr", bufs=2)
                ve.reciprocal(rnr[:], rn[:])
            bc = _pst(ps, [128, C], "mm", 2)
            te.matmul(bc[:], ones_row, rnr[:])
            for gd in range(GD):
                ve.tensor_mul(src[:, gd, :], src[:, gd, :], bc[:])

        def mem_l1(dst_act, src, extra_copy=None):
            """dst_act = silu(mem_w1 @ src) via quarter psum tiles; also
            Copy raw h1 into extra_copy when given."""
            for q in range(GO // 2):
                ph = _pst(ps, [128, 2, C], "h2", 2)
                for gg in range(2):
                    go = q * 2 + gg
                    for gd in range(GD):
                        te.matmul(ph[:, gg, :],
                                  mw1T[:, gd, go * 128:(go + 1) * 128],
                                  src[:, gd, :], start=(gd == 0),
                                  stop=(gd == GD - 1))
                sc.activation(dst_act[:, q * 2:q * 2 + 2, :], ph[:], AF.Silu)
                if extra_copy is not None:
                    sc.activation(extra_copy[:, q * 2:q * 2 + 2, :], ph[:],
                                  AF.Copy)

        # ============ PASS 1: k/v + gradient accumulation ============
        work_cm = tc.tile_pool(name="work", bufs=1)
        work = work_cm.__enter__()
        with tc.tile_pool(name="w_kv", bufs=1) as w_kv:
            load_proj_weights(w_kv, "kv")
            carry_k = proj_halo("k")[:, :, 1:PAD]
            carry_v = proj_halo("v")[:, :, 1:PAD]

            def p1_stage_a(ci, carry_k, carry_v):
                xc = work.tile([128, GD, C], f16, name="xc", tag="xc", bufs=2)
                sy.dma_start(xc[:],
                             d["xT"][:, :, PAD + ci * C:PAD + (ci + 1) * C])
                xs_c = work.tile([128, GD, 1], f32, name="xs_c", tag="xs_c")
                ve.tensor_reduce(xs_c[:], xc[:], mybir.AxisListType.X, ALU.add)
                ve.tensor_add(xsum[:], xsum[:], xs_c[:])
                k_n = work.tile([128, GD, C], f16, name="k_n", tag="k_n",
                                bufs=2)
                y2k = proj_conv_silu("k", xc, k_n, carry_k)
                vs = work.tile([128, GD, C], f16, name="vs", tag="vs", bufs=2)
                y2v = proj_conv_silu("v", xc, vs, carry_v)
                return (k_n, vs), y2k[:, :, C:C + 3], y2v[:, :, C:C + 3]

            def p1_stage_b(st):
                k_n, vs = st
                l2norm_inplace(k_n)
                a1 = work.tile([128, GO, C], f16, name="a1", tag="a1", bufs=2)
                h1s = work.tile([128, GO, C], f16, name="h1s", tag="h1s")
                mem_l1(a1, k_n, extra_copy=h1s)
                sp1 = work.tile([128, GO, C], f16, name="sp1", tag="sp1")
                sc.activation(sp1[:], h1s[:], AF.Derivative_silu)
                dpredu = work.tile([128, GD, C], f16, name="dpredu",
                                   tag="dpredu")
                for q in range(GD // 2):
                    pp = _pst(ps, [128, 2, C], "h2", 2)
                    for gg in range(2):
                        gi = q * 2 + gg
                        for go in range(GO):
                            te.matmul(pp[:, gg, :],
                                      mw2T[:, go, gi * 128:(gi + 1) * 128],
                                      a1[:, go, :], start=(go == 0),
                                      stop=(go == GO - 1))
                    ve.tensor_sub(dpredu[:, q * 2:q * 2 + 2, :], pp[:],
                                  vs[:, q * 2:q * 2 + 2, :])
                dh1 = work.tile([128, GO, C], f16, name="dh1", tag="dh1")
                for q in range(GO // 2):
                    pd = _pst(ps, [128, 2, C], "h2", 2)
                    for gg in range(2):
                        go = q * 2 + gg
                        for gi in range(GD):
                            te.matmul(pd[:, gg, :],
                                      mw2n[:, gi, go * 128:(go + 1) * 128],
                                      dpredu[:, gi, :], start=(gi == 0),
                                      stop=(gi == GD - 1))
                    ve.tensor_mul(dh1[:, q * 2:q * 2 + 2, :], pd[:],
                                  sp1[:, q * 2:q * 2 + 2, :])
                k_tm = work.tile([128, NSUB, DIM], f16, name="k_tm",
                                 tag="k_tm")
                dp_tm = work.tile([128, NSUB, DIM], f16, name="dp_tm",
                                  tag="dp_tm")
                a1_tm = work.tile([128, NSUB, HID], f16, name="a1_tm",
                                  tag="a1_tm")
                dh1_tm = work.tile([128, NSUB, HID], f16, name="dh1_tm",
                                   tag="dh1_tm")
                for sub in range(NSUB):
                    for src, dst, ng in ((k_n, k_tm, GD), (dpredu, dp_tm, GD),
                                         (a1, a1_tm, GO), (dh1, dh1_tm, GO)):
                        for h in range(ng // 4):
                            tp = _pst(ps, [128, 512], "sm", 2, f16)
                            for j in range(4):
                                g = h * 4 + j
                                te.transpose(
                                    tp[:, j * 128:(j + 1) * 128],
                                    src[:, g, sub * 128:(sub + 1) * 128],
                                    ident[:])
                            sc.activation(dst[:, sub, h * 512:(h + 1) * 512],
                                          tp[:], AF.Copy)
                for go in range(GO):
                    p = _pst(ps, [128, DIM], "sm", 2)
                    for sub in range(NSUB):
                        te.matmul(p[:],
                                  dh1_tm[:, sub, go * 128:(go + 1) * 128],
                                  k_tm[:, sub, :], start=(sub == 0),
                                  stop=(sub == NSUB - 1))
                    ve.scalar_tensor_tensor(g1acc[:, go, :], p[:], C_LOSS,
                                            g1acc[:, go, :], ALU.mult,
                                            ALU.add)
                for gi in range(GD):
                    for h in range(2):
                        p = _pst(ps, [128, 512], "sm", 2)
                        for sub in range(NSUB):
                            te.matmul(p[:],
                                      dp_tm[:, sub, gi * 128:(gi + 1) * 128],
                                      a1_tm[:, sub, h * 512:(h + 1) * 512],
                                      start=(sub == 0), stop=(sub == NSUB - 1))
                        ve.scalar_tensor_tensor(
                            g2acc[:, gi, h * 512:(h + 1) * 512], p[:], C_LOSS,
                            g2acc[:, gi, h * 512:(h + 1) * 512],
                            ALU.mult, ALU.add)

            prev1 = None
            for ci in range(NCH):
                cur1, carry_k, carry_v = p1_stage_a(ci, carry_k, carry_v)
                if prev1 is not None:
                    p1_stage_b(prev1)
                prev1 = cur1
            p1_stage_b(prev1)

        # ---- launch collectives (overlap with pass 2) ----
        part = work.tile([128, GD, B], f32, name="part", tag="part")
        for bcol in range(B):
            ve.tensor_scalar_mul(part[:, :, bcol:bcol + 1], xsum[:],
                                 sel[:, bcol:bcol + 1])
        sy.dma_start(d["cc_x_in"][:].rearrange("p (g b) -> p g b", g=GD),
                     part[:])
        gp.collective_compute("AllReduce", ALU.add, replica_groups=REPLICA,
                              ins=[d["cc_x_in"][:]], outs=[d["cc_x_out"][:]])
        g1v = d["cc_g_in"][:, 0:SL1].rearrange("s (p dd) -> p s dd", p=128)
        sy.dma_start(g1v, g1acc[:])
        g2v = d["cc_g_in"][:, SL1:].rearrange(
            "(gi ph) (q o) -> ph q gi o", gi=GD, q=64)
        for ph in range(2):
            sy.dma_start(g2v[ph], g2acc[ph * 64:(ph + 1) * 64, :, :])
        gp.collective_compute("ReduceScatter", ALU.add, replica_groups=REPLICA,
                              ins=[d["cc_g_in"][:]], outs=[d["cc_g_out"][:]])

        # ============ PASS 2: queries -> retrieved -> output ============
        with tc.tile_pool(name="w_q", bufs=1) as w_q:
            load_proj_weights(w_q, "q")
            woutT = w_q.tile([128, GD, DIM], f16, name="woutT")
            sy.dma_start(woutT[:], d["woutT_d"][:])
            carry_q = proj_halo("q")[:, :, 1:PAD]

            def p2_stage_a(ci, carry_q):
                xc = work.tile([128, GD, C], f16, name="xc", tag="xc", bufs=2)
                sy.dma_start(xc[:],
                             d["xT"][:, :, PAD + ci * C:PAD + (ci + 1) * C])
                q_n = work.tile([128, GD, C], f16, name="q_n", tag="k_n",
                                bufs=2)
                y2q = proj_conv_silu("q", xc, q_n, carry_q)
                return (ci, q_n), y2q[:, :, C:C + 3]

            def p2_stage_b(st):
                ci, q_n = st
                l2norm_inplace(q_n)
                aq = work.tile([128, GO, C], f16, name="aq", tag="a1", bufs=2)
                mem_l1(aq, q_n)
                r_s = work.tile([128, GD, C], f16, name="r_s", tag="dpredu")
                for q in range(GD // 2):
                    pr = _pst(ps, [128, 2, C], "h2", 2)
                    for gg in range(2):
                        gi = q * 2 + gg
                        for go in range(GO):
                            te.matmul(pr[:, gg, :],
                                      mw2T[:, go, gi * 128:(gi + 1) * 128],
                                      aq[:, go, :], start=(go == 0),
                                      stop=(go == GO - 1))
                    sc.activation(r_s[:, q * 2:q * 2 + 2, :], pr[:], AF.Copy)
                for q in range(GD // 2):
                    outs = work.tile([128, 2, C], f32, name="outs",
                                     tag="outs", bufs=2)
                    po = _pst(ps, [128, 2, C], "h2", 2)
                    for gg in range(2):
                        gu = q * 2 + gg
                        for gi in range(GD):
                            te.matmul(po[:, gg, :],
                                      woutT[:, gi, gu * 128:(gu + 1) * 128],
                                      r_s[:, gi, :], start=(gi == 0),
                                      stop=(gi == GD - 1))
                    sc.activation(outs[:], po[:], AF.Copy)
                    sy.dma_start(
                        d["outT"][:, q * 2:q * 2 + 2, ci * C:(ci + 1) * C],
                        outs[:])

            prev2 = None
            for ci in range(NCH):
                cur2, carry_q = p2_stage_a(ci, carry_q)
                if prev2 is not None:
                    p2_stage_b(prev2)
                prev2 = cur2
            p2_stage_b(prev2)

        work_cm.__exit__(None, None, None)

        # ============ tail: gates, momentum/decay update ============
        with tc.tile_pool(name="tail", bufs=1) as tail:
            gwT = tail.tile([128, GD, 3, DIM], f32, name="gwT")
            sy.dma_start(gwT[:], d["gwT_d"][:])
            gateb = tail.tile([128, GD, 3], f32, name="gateb")
            sy.dma_start(gateb[:], d["gateb_d"][:])
            xsb = tail.tile([128, GD, B], f32, name="xsb")
            sy.dma_start(xsb[:],
                         d["cc_x_out"][:].rearrange("p (g b) -> p g b", g=GD))
            gsc = tail.tile([1, 3], f32, name="gsc")
            for w in range(3):
                sig = tail.tile([128, GD, B], f32, name=f"sig{w}", tag="sig")
                for gj in range(GD):
                    p = _pst(ps, [128, B], "mm", 2)
                    for gd in range(GD):
                        te.matmul(p[:], gwT[:, gd, w, gj * 128:(gj + 1) * 128],
                                  xsb[:, gd, :], start=(gd == 0),
                                  stop=(gd == GD - 1))
                    sc.activation(sig[:, gj, :], p[:], AF.Sigmoid,
                                  bias=gateb[:, gj, w:w + 1], scale=1.0 / S)
                srow = tail.tile([128, 1], f32, name=f"srow{w}", tag="srow")
                ve.tensor_reduce(srow[:], sig[:], mybir.AxisListType.XY,
                                 ALU.add)
                p = _pst(ps, [1, 1], "mm", 2)
                te.matmul(p[:], ones_col32[:], srow[:])
                sc.activation(gsc[0:1, w:w + 1], p[:], AF.Copy,
                              scale=1.0 / (DIM * B))
            ones_row32 = tail.tile([1, 128], f32, name="ones_row32")
            gp.memset(ones_row32[:], 1.0)
            gbp = _pst(ps, [128, 3], "mm", 2)
            te.matmul(gbp[:], ones_row32[:], gsc[:])
            gb = tail.tile([128, 3], f32, name="gb")
            sc.activation(gb[:], gbp[:], AF.Copy)
            oneminus = tail.tile([128, 1], f32, name="oneminus")
            ve.tensor_scalar(oneminus[:], gb[:, 0:1], -1.0, 1.0, ALU.mult,
                             ALU.add)
            theta = tail.tile([128, 1], f32, name="theta")
            ve.tensor_scalar_mul(theta[:], gb[:, 1:2], MEM_LR)
            eta = tail.tile([128, 1], f32, name="eta")
            ve.tensor_scalar_mul(eta[:], gb[:, 2:3], MEM_MOM)

            g1s = tail.tile([128, DIM], f32, name="g1s")
            sy.dma_start(g1s[:],
                         d["cc_g_out"][0:SL1].rearrange("(p dd) -> p dd",
                                                        p=128))
            g2s = tail.tile([64, HID], f32, name="g2s")
            sy.dma_start(g2s[:],
                         d["cc_g_out"][SL1:].rearrange("(q o) -> q o", q=64))
            mw1s = tail.tile([128, DIM], f32, name="mw1s")
            sy.dma_start(mw1s[:], d["mw1s_d"][:])
            mom1s = tail.tile([128, DIM], f32, name="mom1s")
            sy.dma_start(mom1s[:], d["mom1s_d"][:])
            mw2s = tail.tile([64, HID], f32, name="mw2s")
            sy.dma_start(mw2s[:], d["mw2s_d"][:])
            mom2s = tail.tile([64, HID], f32, name="mom2s")
            sy.dma_start(mom2s[:], d["mom2s_d"][:])

            for gsl, mws, moms, s_out, nw_out, np_ in (
                    (g1s, mw1s, mom1s, d["s1_s"], d["nw1_s"], 128),
                    (g2s, mw2s, mom2s, d["s2_s"], d["nw2_s"], 64)):
                t1 = tail.tile(list(gsl.shape), f32, name="t1", tag="t1")
                ve.tensor_scalar_mul(t1[:], gsl[:], theta[0:np_, :])
                st = tail.tile(list(gsl.shape), f32, name="st", tag="st")
                ve.scalar_tensor_tensor(st[:], moms[:], eta[0:np_, :], t1[:],
                                        ALU.mult, ALU.subtract)
                nwt = tail.tile(list(gsl.shape), f32, name="nwt", tag="nwt")
                ve.scalar_tensor_tensor(nwt[:], mws[:], oneminus[0:np_, :],
                                        st[:], ALU.mult, ALU.add)
                sy.dma_start(s_out[:], st[:])
                sy.dma_start(nw_out[:], nwt[:])


# ======================= host side =======================

def _gT(w, dt=np.float16):
    """(out, in=512) torch-Linear weight -> grouped lhsT [128, 4, out]."""
    return np.ascontiguousarray(
        w.T.reshape(GD, 128, -1).transpose(1, 0, 2)).astype(dt)


def _g_natural(w, groups, dt=np.float16):
    """(rows, cols) -> grouped [128, groups, cols] (rows on partitions)."""
    return np.ascontiguousarray(
        w.reshape(groups, 128, -1).transpose(1, 0, 2)).astype(dt)


def _prep(inputs):
    x = np.asarray(inputs["x"], np.float32)
    shared = {}
    for t, (a, b_) in {"k": ("w_k1", "w_k2"), "v": ("w_v1", "w_v2"),
                       "q": ("w_q1", "w_q2")}.items():
        shared[f"w1T_{t}"] = _gT(np.asarray(inputs[a], np.float32))
        shared[f"w2T_{t}"] = _gT(np.asarray(inputs[b_], np.float32))
    mem_w1 = np.asarray(inputs["mem_w1"], np.float32)
    mem_w2 = np.asarray(inputs["mem_w2"], np.float32)
    shared["mw1T"] = _gT(mem_w1)                       # [128,4,1024]
    shared["mw2T"] = _g_natural(mem_w2.T, GO)          # [128,8,512]
    shared["mw2n"] = _g_natural(mem_w2, GD)            # [128,4,1024]
    shared["woutT"] = _gT(np.asarray(inputs["w_out"], np.float32))
    gw = np.stack([_gT(np.asarray(inputs[f"gate_{t}_w"], np.float32),
                       np.float32)
                   for t in ("d", "lr", "m")], axis=2)  # [128,4,3,512]
    shared["gwT"] = np.ascontiguousarray(gw)
    shared["ident"] = np.eye(128, dtype=np.float16)
    shared["ones"] = np.ones((128, 129), np.float16)
    cw = np.stack([np.asarray(inputs[f"conv_{t}_w"], np.float32)[:, 0, :]
                   for t in ("k", "v", "q")], axis=1)   # (512, 3, 4)
    shared["convw"] = np.ascontiguousarray(
        cw.reshape(GD, 128, 3, K).transpose(1, 0, 2, 3))
    cb = np.stack([np.asarray(inputs[f"conv_{t}_b"], np.float32)
                   for t in ("k", "v", "q")], axis=1)   # (512, 3)
    shared["convb"] = np.ascontiguousarray(
        cb.reshape(GD, 128, 3).transpose(1, 0, 2))
    gb = np.stack([np.asarray(inputs[f"gate_{t}_b"], np.float32)
                   for t in ("d", "lr", "m")], axis=1)  # (512, 3)
    shared["gateb"] = np.ascontiguousarray(
        gb.reshape(GD, 128, 3).transpose(1, 0, 2))
    mom1 = np.asarray(inputs["mom1"], np.float32)
    mom2 = np.asarray(inputs["mom2"], np.float32)

    in_maps = []
    for c in range(N_CORES):
        b = c // 2
        half = c % 2
        seq = x[b]
        if half == 0:
            seg = np.concatenate(
                [np.zeros((PAD, DIM), np.float32), seq[0:T]], axis=0)
        else:
            seg = seq[T - PAD:2 * T]
        xTg = np.ascontiguousarray(
            seg.T.reshape(GD, 128, T + PAD).transpose(1, 0, 2)
        ).astype(np.float16)
        sel = np.zeros((128, B), np.float32)
        sel[:, b] = 1.0
        m = dict(shared)
        m["xT"] = xTg
        m["sel"] = sel
        m["mw1s"] = np.ascontiguousarray(mem_w1[c * 128:(c + 1) * 128])
        m["mom1s"] = np.ascontiguousarray(mom1[c * 128:(c + 1) * 128])
        m["mw2s"] = np.ascontiguousarray(mem_w2[c * 64:(c + 1) * 64])
        m["mom2s"] = np.ascontiguousarray(mom2[c * 64:(c + 1) * 64])
        in_maps.append(m)
    return in_maps


def _unshard(results):
    outs = []
    for c in range(N_CORES):
        a = results[c]["outT"]          # [128, 4, 4096]
        outs.append(np.moveaxis(a, 1, 0).reshape(DIM, T).T)
    output = np.concatenate(outs, axis=0).reshape(B, S, DIM)
    nw1 = np.concatenate([results[c]["nw1_s"] for c in range(N_CORES)], axis=0)
    s1 = np.concatenate([results[c]["s1_s"] for c in range(N_CORES)], axis=0)
    nw2 = np.concatenate([results[c]["nw2_s"] for c in range(N_CORES)], axis=0)
    s2 = np.concatenate([results[c]["s2_s"] for c in range(N_CORES)], axis=0)
    return (output, nw1, nw2, s1, s2)


def run(in_maps, trace=False):
    if "nc" not in _CACHE:
        _CACHE["nc"] = _build()
    return run_bass_kernel_spmd(_CACHE["nc"], in_maps,
                                core_ids=list(range(N_CORES)), trace=trace)


def kernel(**inputs):
    res = run(_prep(inputs))
    return _unshard(res.results)


if __name__ == "__main__":
    import reference as R
    inp = {k: np.asarray(v) for k, v in R.setup_inputs().items()}
    got = kernel(**inp)
    print([g.shape for g in got])


# revision 21
# speedup vs baseline: 1.7308x; 1.1135x over previous
"""Trainium2 Bass kernel for nn_NeuralLongTermMemory.

Sharding: tokens (B*S = 32768) split 8 ways -> 4096 tokens/core (half a
batch-sequence each).  All weights replicated.  Gradients of the memory
MLP are partial-summed per core and combined with one ReduceScatter; the
data-dependent scalar gates use one small AllReduce of per-batch x sums.

Layout: feature-major - features on SBUF partitions (grouped
[128, G, cols]), tokens on the free dimension.  The causal depthwise
conv is shifted-window multiply-adds on the free dim with a 3-column
carry between chunks.  Gradient outer-products need token-major
operands, produced with PE transposes.

Matmul operands are fp16 (fp32 PSUM accumulation); gradient
accumulators, gates, and outputs stay fp32.
"""

import numpy as np
import concourse.bass as bass
import concourse.mybir as mybir
import concourse.tile as tile
from concourse import bacc
from concourse.bass_utils import run_bass_kernel_spmd

f32 = mybir.dt.float32
f16 = mybir.dt.float16
AF = mybir.ActivationFunctionType
ALU = mybir.AluOpType

B, S, DIM, HID, K = 4, 8192, 512, 1024, 4
N_CORES = 8
T = B * S // N_CORES        # 4096 tokens per core
PAD = 4                     # x halo columns at the sequence head
C = 512                     # chunk tokens
NCH = T // C
NSUB = C // 128
GD = DIM // 128             # 4
GO = HID // 128             # 8
C_LOSS = 2.0 / (B * S * DIM)
MEM_LR, MEM_MOM = 0.01, 0.9
EPS = 1e-12
SL1 = 128 * 512             # g1 slice elems per core
REPLICA = [list(range(N_CORES))]

_CACHE = {}


def _pst(ps, shape, tag, bufs, dt=f32):
    return ps.tile(shape, dt, name=tag, tag=tag, bufs=bufs, space="PSUM")


def _build():
    nc = bacc.Bacc("TRN2", target_bir_lowering=False, debug=False,
                   num_devices=N_CORES)

    def din(name, shape, dt=f16):
        return nc.dram_tensor(name, shape, dt, kind="ExternalInput")

    def dout(name, shape):
        return nc.dram_tensor(name, shape, f32, kind="ExternalOutput")

    xT = din("xT", [128, GD, T + PAD])
    w1T = {t: din(f"w1T_{t}", [128, GD, DIM]) for t in "kvq"}
    w2T = {t: din(f"w2T_{t}", [128, GD, DIM]) for t in "kvq"}
    mw1T_d = din("mw1T", [128, GD, HID])
    mw2T_d = din("mw2T", [128, GO, DIM])
    mw2n_d = din("mw2n", [128, GD, HID])
    woutT_d = din("woutT", [128, GD, DIM])
    gwT_d = din("gwT", [128, GD, 3, DIM], f32)
    ident_d = din("ident", [128, 128])
    ones_d = din("ones", [128, 129])
    convw_d = din("convw", [128, GD, 3, K], f32)
    convb_d = din("convb", [128, GD, 3], f32)
    gateb_d = din("gateb", [128, GD, 3], f32)
    sel_d = din("sel", [128, B], f32)
    mw1s_d = din("mw1s", [128, DIM], f32)
    mom1s_d = din("mom1s", [128, DIM], f32)
    mw2s_d = din("mw2s", [64, HID], f32)
    mom2s_d = din("mom2s", [64, HID], f32)

    outT = dout("outT", [128, GD, T])
    nw1_s = dout("nw1_s", [128, DIM])
    s1_s = dout("s1_s", [128, DIM])
    nw2_s = dout("nw2_s", [64, HID])
    s2_s = dout("s2_s", [64, HID])

    cc_g_in = nc.dram_tensor("cc_g_in", [N_CORES, 2 * SL1], f32)
    cc_g_out = nc.dram_tensor("cc_g_out", [2 * SL1], f32)
    cc_x_in = nc.dram_tensor("cc_x_in", [128, GD * B], f32)
    cc_x_out = nc.dram_tensor("cc_x_out", [128, GD * B], f32,
                              addr_space="Shared")

    with tile.TileContext(nc) as tc:
        _emit(nc, tc, locals())
    nc.compile()
    return nc


def _emit(nc, tc, d):
    sc, ve, te, sy, gp = nc.scalar, nc.vector, nc.tensor, nc.sync, nc.gpsimd

    with (
        tc.tile_pool(name="consts", bufs=1) as consts,
        tc.tile_pool(name="accs", bufs=1) as accs,
        tc.tile_pool(name="wmem", bufs=1) as wmem,
        tc.tile_pool(name="ps", bufs=1, space="PSUM") as ps,
    ):
        # ---- constants / resident weights ----
        ident = consts.tile([128, 128], f16)
        sy.dma_start(ident[:], d["ident_d"][:])
        onesb = consts.tile([128, 129], f16)
        sy.dma_start(onesb[:], d["ones_d"][:])
        ones_col = onesb[:, 0:1]
        ones_row = onesb[0:1, 1:129]
        ones_col32 = consts.tile([128, 1], f32)
        gp.memset(ones_col32[:], 1.0)
        convw = consts.tile([128, GD, 3, K], f32)
        sy.dma_start(convw[:], d["convw_d"][:])
        convb = consts.tile([128, GD, 3], f32)
        sy.dma_start(convb[:], d["convb_d"][:])
        sel = consts.tile([128, B], f32)
        sy.dma_start(sel[:], d["sel_d"][:])

        mw1T = wmem.tile([128, GD, HID], f16)
        sy.dma_start(mw1T[:], d["mw1T_d"][:])
        mw2T = wmem.tile([128, GO, DIM], f16)
        sy.dma_start(mw2T[:], d["mw2T_d"][:])
        mw2n = wmem.tile([128, GD, HID], f16)
        sy.dma_start(mw2n[:], d["mw2n_d"][:])

        g1acc = accs.tile([128, GO, DIM], f32)
        gp.memset(g1acc[:], 0.0)
        g2acc = accs.tile([128, GD, HID], f32)
        gp.memset(g2acc[:], 0.0)
        xsum = accs.tile([128, GD, 1], f32)
        gp.memset(xsum[:], 0.0)

        w1sb = {}
        w2sb = {}

        def load_proj_weights(pool, tensors):
            for t in tensors:
                wa = pool.tile([128, GD, DIM], f16, name=f"w1sb_{t}",
                               tag=f"w1sb_{t}")
                sy.dma_start(wa[:], d["w1T"][t][:])
                w1sb[t] = wa
                wb = pool.tile([128, GD, DIM], f16, name=f"w2sb_{t}",
                               tag=f"w2sb_{t}")
                sy.dma_start(wb[:], d["w2T"][t][:])
                w2sb[t] = wb

        def proj_halo(tn):
            """proj2(silu(proj1(x))) for the 4 head-halo tokens."""
            xch = work.tile([128, GD, PAD], f16, name="xch", tag="xch")
            sy.dma_start(xch[:], d["xT"][:, :, 0:PAD])
            y1hs = work.tile([128, GD, PAD], f16, name="y1hs", tag="y1hs")
            for gj in range(GD):
                p = _pst(ps, [128, PAD], "mm", 2)
                for gd in range(GD):
                    te.matmul(p[:], w1sb[tn][:, gd, gj * 128:(gj + 1) * 128],
                              xch[:, gd, :], start=(gd == 0),
                              stop=(gd == GD - 1))
                sc.activation(y1hs[:, gj, :], p[:], AF.Silu)
            yh = work.tile([128, GD, PAD], f16, name=f"yh{tn}", tag=f"yh_{tn}")
            for gj in range(GD):
                p = _pst(ps, [128, PAD], "mm", 2)
                for gd in range(GD):
                    te.matmul(p[:], w2sb[tn][:, gd, gj * 128:(gj + 1) * 128],
                              y1hs[:, gd, :], start=(gd == 0),
                              stop=(gd == GD - 1))
                sc.activation(yh[:, gj, :], p[:], AF.Copy)
            return yh

        def proj_conv_silu(tn, xc, out_t, carry):
            """proj2(silu(proj1(x))) -> y2 (sbuf, 3-col carry prefix) ->
            causal conv -> +bias -> silu -> out_t.  Returns y2 (next
            chunk's carry source)."""
            ti = "kvq".index(tn)
            y1s = work.tile([128, GD, C], f16, name="y1s", tag="y1s")
            for gj in range(GD):
                p = _pst(ps, [128, C], "mm", 2)
                for gd in range(GD):
                    te.matmul(p[:], w1sb[tn][:, gd, gj * 128:(gj + 1) * 128],
                              xc[:, gd, :], start=(gd == 0),
                              stop=(gd == GD - 1))
                sc.activation(y1s[:, gj, :], p[:], AF.Silu)
            y2tag = "y2_a" if tn in "kq" else "y2_b"
            y2 = work.tile([128, GD, C + 3], f16, name=f"y2{tn}",
                           tag=y2tag, bufs=2)
            ve.tensor_copy(y2[:, :, 0:3], carry)
            for gj in range(GD):
                p = _pst(ps, [128, C], "mm", 2)
                for gd in range(GD):
                    te.matmul(p[:], w2sb[tn][:, gd, gj * 128:(gj + 1) * 128],
                              y1s[:, gd, :], start=(gd == 0),
                              stop=(gd == GD - 1))
                sc.activation(y2[:, gj, 3:C + 3], p[:], AF.Copy)
            ca = work.tile([128, GD, C], f16, name="ca", tag="ca")
            for gj in range(GD):
                ve.tensor_scalar(ca[:, gj, :], y2[:, gj, 0:C],
                                 convw[:, gj, ti, 0:1], None, ALU.mult)
                for kk in range(1, K):
                    ve.scalar_tensor_tensor(ca[:, gj, :], y2[:, gj, kk:kk + C],
                                            convw[:, gj, ti, kk:kk + 1],
                                            ca[:, gj, :], ALU.mult, ALU.add)
            for gj in range(GD):
                sc.activation(out_t[:, gj, :], ca[:, gj, :], AF.Silu,
                              bias=convb[:, gj, ti:ti + 1])
            return y2

        def l2norm_inplace(src):
            sq = work.tile([128, GD, C], f16, name="sq", tag="sq")
            sc.activation(sq[:], src[:], AF.Square)
            ssp = _pst(ps, [1, C], "mm", 2)
            for gd in range(GD):
                te.matmul(ssp[:], ones_col, sq[:, gd, :],
                          start=(gd == 0), stop=(gd == GD - 1))
            rn = work.tile([1, C], f32, name="rn", tag="rn", bufs=2)
            sc.activation(rn[:], ssp[:], AF.Sqrt)
            ve.tensor_scalar_max(rn[:], rn[:], EPS)
            with nc.allow_low_precision("norm factor to f16"):
                rnr = work.tile([1, C], f16, name="rnr", tag="rnr", bufs=2)
                ve.reciprocal(rnr[:], rn[:])
            bc = _pst(ps, [128, C], "mm", 2)
            te.matmul(bc[:], ones_row, rnr[:])
            for gd in range(GD):
                ve.tensor_mul(src[:, gd, :], src[:, gd, :], bc[:])

        def mem_l1(dst_act, src, extra_copy=None):
            """dst_act = silu(mem_w1 @ src) via quarter psum tiles; also
            Copy raw h1 into extra_copy when given."""
            for q in range(GO // 2):
                ph = _pst(ps, [128, 2, C], "h2", 2)
                for gg in range(2):
                    go = q * 2 + gg
                    for gd in range(GD):
                        te.matmul(ph[:, gg, :],
                                  mw1T[:, gd, go * 128:(go + 1) * 128],
                                  src[:, gd, :], start=(gd == 0),
                                  stop=(gd == GD - 1))
                sc.activation(dst_act[:, q * 2:q * 2 + 2, :], ph[:], AF.Silu)
                if extra_copy is not None:
                    sc.activation(extra_copy[:, q * 2:q * 2 + 2, :], ph[:],
                                  AF.Copy)

        # ============ PASS 1: k/v + gradient accumulation ============
        work_cm = tc.tile_pool(name="work", bufs=1)
        work = work_cm.__enter__()
        with tc.tile_pool(name="w_kv", bufs=1) as w_kv:
            load_proj_weights(w_kv, "kv")
            carry_k = proj_halo("k")[:, :, 1:PAD]
            carry_v = proj_halo("v")[:, :, 1:PAD]

            def p1_stage_a(ci, carry_k, carry_v):
                xc = work.tile([128, GD, C], f16, name="xc", tag="xc", bufs=2)
                sy.dma_start(xc[:],
                             d["xT"][:, :, PAD + ci * C:PAD + (ci + 1) * C])
                xs_c = work.tile([128, GD, 1], f32, name="xs_c", tag="xs_c")
                ve.tensor_reduce(xs_c[:], xc[:], mybir.AxisListType.X, ALU.add)
                ve.tensor_add(xsum[:], xsum[:], xs_c[:])
                k_n = work.tile([128, GD, C], f16, name="k_n", tag="k_n",
                                bufs=2)
                y2k = proj_conv_silu("k", xc, k_n, carry_k)
                vs = work.tile([128, GD, C], f16, name="vs", tag="vs", bufs=2)
                y2v = proj_conv_silu("v", xc, vs, carry_v)
                return (k_n, vs), y2k[:, :, C:C + 3], y2v[:, :, C:C + 3]

            def p1_stage_b(st):
                k_n, vs = st
                a1 = work.tile([128, GO, C], f16, name="a1", tag="a1", bufs=2)
                h1s = work.tile([128, GO, C], f16, name="h1s", tag="h1s")
                mem_l1(a1, k_n, extra_copy=h1s)
                sp1 = work.tile([128, GO, C], f16, name="sp1", tag="sp1")
                sc.activation(sp1[:], h1s[:], AF.Derivative_silu)
                dpredu = work.tile([128, GD, C], f16, name="dpredu",
                                   tag="dpredu")
                for q in range(GD // 2):
                    pp = _pst(ps, [128, 2, C], "h2", 2)
                    for gg in range(2):
                        gi = q * 2 + gg
                        for go in range(GO):
                            te.matmul(pp[:, gg, :],
                                      mw2T[:, go, gi * 128:(gi + 1) * 128],
                                      a1[:, go, :], start=(go == 0),
                                      stop=(go == GO - 1))
                    ve.tensor_sub(dpredu[:, q * 2:q * 2 + 2, :], pp[:],
                                  vs[:, q * 2:q * 2 + 2, :])
                dh1 = work.tile([128, GO, C], f16, name="dh1", tag="dh1")
                for q in range(GO // 2):
                    pd = _pst(ps, [128, 2, C], "h2", 2)
                    for gg in range(2):
                        go = q * 2 + gg
                        for gi in range(GD):
                            te.matmul(pd[:, gg, :],
                                      mw2n[:, gi, go * 128:(go + 1) * 128],
                                      dpredu[:, gi, :], start=(gi == 0),
                                      stop=(gi == GD - 1))
                    ve.tensor_mul(dh1[:, q * 2:q * 2 + 2, :], pd[:],
                                  sp1[:, q * 2:q * 2 + 2, :])
                k_tm = work.tile([128, NSUB, DIM], f16, name="k_tm",
                                 tag="k_tm")
                dp_tm = work.tile([128, NSUB, DIM], f16, name="dp_tm",
                                  tag="dp_tm")
                a1_tm = work.tile([128, NSUB, HID], f16, name="a1_tm",
                                  tag="a1_tm")
                dh1_tm = work.tile([128, NSUB, HID], f16, name="dh1_tm",
                                   tag="dh1_tm")
                for sub in range(NSUB):
                    for src, dst, ng in ((k_n, k_tm, GD), (dpredu, dp_tm, GD),
                                         (a1, a1_tm, GO), (dh1, dh1_tm, GO)):
                        for h in range(ng // 4):
                            tp = _pst(ps, [128, 512], "sm", 2, f16)
                            for j in range(4):
                                g = h * 4 + j
                                te.transpose(
                                    tp[:, j * 128:(j + 1) * 128],
                                    src[:, g, sub * 128:(sub + 1) * 128],
                                    ident[:])
                            sc.activation(dst[:, sub, h * 512:(h + 1) * 512],
                                          tp[:], AF.Copy)
                for go in range(GO):
                    p = _pst(ps, [128, DIM], "sm", 2)
                    for sub in range(NSUB):
                        te.matmul(p[:],
                                  dh1_tm[:, sub, go * 128:(go + 1) * 128],
                                  k_tm[:, sub, :], start=(sub == 0),
                                  stop=(sub == NSUB - 1))
                    ve.scalar_tensor_tensor(g1acc[:, go, :], p[:], C_LOSS,
                                            g1acc[:, go, :], ALU.mult,
                                            ALU.add)
                for gi in range(GD):
                    for h in range(2):
                        p = _pst(ps, [128, 512], "sm", 2)
                        for sub in range(NSUB):
                            te.matmul(p[:],
                                      dp_tm[:, sub, gi * 128:(gi + 1) * 128],
                                      a1_tm[:, sub, h * 512:(h + 1) * 512],
                                      start=(sub == 0), stop=(sub == NSUB - 1))
                        ve.scalar_tensor_tensor(
                            g2acc[:, gi, h * 512:(h + 1) * 512], p[:], C_LOSS,
                            g2acc[:, gi, h * 512:(h + 1) * 512],
                            ALU.mult, ALU.add)

            prev1 = None
            for ci in range(NCH):
                cur1, carry_k, carry_v = p1_stage_a(ci, carry_k, carry_v)
                l2norm_inplace(cur1[0])
                if prev1 is not None:
                    p1_stage_b(prev1)
                prev1 = cur1
            p1_stage_b(prev1)

        # ---- launch collectives (overlap with pass 2) ----
        part = work.tile([128, GD, B], f32, name="part", tag="part")
        for bcol in range(B):
            ve.tensor_scalar_mul(part[:, :, bcol:bcol + 1], xsum[:],
                                 sel[:, bcol:bcol + 1])
        sy.dma_start(d["cc_x_in"][:].rearrange("p (g b) -> p g b", g=GD),
                     part[:])
        gp.collective_compute("AllReduce", ALU.add, replica_groups=REPLICA,
                              ins=[d["cc_x_in"][:]], outs=[d["cc_x_out"][:]])
        g1v = d["cc_g_in"][:, 0:SL1].rearrange("s (p dd) -> p s dd", p=128)
        sy.dma_start(g1v, g1acc[:])
        g2v = d["cc_g_in"][:, SL1:].rearrange(
            "(gi ph) (q o) -> ph q gi o", gi=GD, q=64)
        for ph in range(2):
            sy.dma_start(g2v[ph], g2acc[ph * 64:(ph + 1) * 64, :, :])
        gp.collective_compute("ReduceScatter", ALU.add, replica_groups=REPLICA,
                              ins=[d["cc_g_in"][:]], outs=[d["cc_g_out"][:]])

        # ============ PASS 2: queries -> retrieved -> output ============
        with tc.tile_pool(name="w_q", bufs=1) as w_q:
            load_proj_weights(w_q, "q")
            woutT = w_q.tile([128, GD, DIM], f16, name="woutT")
            sy.dma_start(woutT[:], d["woutT_d"][:])
            carry_q = proj_halo("q")[:, :, 1:PAD]

            def p2_stage_a(ci, carry_q):
                xc = work.tile([128, GD, C], f16, name="xc", tag="xc", bufs=2)
                sy.dma_start(xc[:],
                             d["xT"][:, :, PAD + ci * C:PAD + (ci + 1) * C])
                q_n = work.tile([128, GD, C], f16, name="q_n", tag="k_n",
                                bufs=2)
                y2q = proj_conv_silu("q", xc, q_n, carry_q)
                return (ci, q_n), y2q[:, :, C:C + 3]

            def p2_stage_b(st):
                ci, q_n = st
                aq = work.tile([128, GO, C], f16, name="aq", tag="a1", bufs=2)
                mem_l1(aq, q_n)
                r_s = work.tile([128, GD, C], f16, name="r_s", tag="dpredu")
                for q in range(GD // 2):
                    pr = _pst(ps, [128, 2, C], "h2", 2)
                    for gg in range(2):
                        gi = q * 2 + gg
                        for go in range(GO):
                            te.matmul(pr[:, gg, :],
                                      mw2T[:, go, gi * 128:(gi + 1) * 128],
                                      aq[:, go, :], start=(go == 0),
                                      stop=(go == GO - 1))
                    sc.activation(r_s[:, q * 2:q * 2 + 2, :], pr[:], AF.Copy)
                for q in range(GD // 2):
                    outs = work.tile([128, 2, C], f32, name="outs",
                                     tag="outs", bufs=2)
                    po = _pst(ps, [128, 2, C], "h2", 2)
                    for gg in range(2):
                        gu = q * 2 + gg
                        for gi in range(GD):
                            te.matmul(po[:, gg, :],
                                      woutT[:, gi, gu * 128:(gu + 1) * 128],
                                      r_s[:, gi, :], start=(gi == 0),
                                      stop=(gi == GD - 1))
                    sc.activation(outs[:], po[:], AF.Copy)
                    sy.dma_start(
                        d["outT"][:, q * 2:q * 2 + 2, ci * C:(ci + 1) * C],
                        outs[:])

            prev2 = None
            for ci in range(NCH):
                cur2, carry_q = p2_stage_a(ci, carry_q)
                l2norm_inplace(cur2[1])
                if prev2 is not None:
                    p2_stage_b(prev2)
                prev2 = cur2
            p2_stage_b(prev2)

        work_cm.__exit__(None, None, None)

        # ============ tail: gates, momentum/decay update ============
        with tc.tile_pool(name="tail", bufs=1) as tail:
            gwT = tail.tile([128, GD, 3, DIM], f32, name="gwT")
            sy.dma_start(gwT[:], d["gwT_d"][:])
            gateb = tail.tile([128, GD, 3], f32, name="gateb")
            sy.dma_start(gateb[:], d["gateb_d"][:])
            xsb = tail.tile([128, GD, B], f32, name="xsb")
            sy.dma_start(xsb[:],
                         d["cc_x_out"][:].rearrange("p (g b) -> p g b", g=GD))
            gsc = tail.tile([1, 3], f32, name="gsc")
            for w in range(3):
                sig = tail.tile([128, GD, B], f32, name=f"sig{w}", tag="sig")
                for gj in range(GD):
                    p = _pst(ps, [128, B], "mm", 2)
                    for gd in range(GD):
                        te.matmul(p[:], gwT[:, gd, w, gj * 128:(gj + 1) * 128],
                                  xsb[:, gd, :], start=(gd == 0),
                                  stop=(gd == GD - 1))
                    sc.activation(sig[:, gj, :], p[:], AF.Sigmoid,
                                  bias=gateb[:, gj, w:w + 1], scale=1.0 / S)
                srow = tail.tile([128, 1], f32, name=f"srow{w}", tag="srow")
                ve.tensor_reduce(srow[:], sig[:], mybir.AxisListType.XY,
                                 ALU.add)
                p = _pst(ps, [1, 1], "mm", 2)
                te.matmul(p[:], ones_col32[:], srow[:])
                sc.activation(gsc[0:1, w:w + 1], p[:], AF.Copy,
                              scale=1.0 / (DIM * B))
            ones_row32 = tail.tile([1, 128], f32, name="ones_row32")
            gp.memset(ones_row32[:], 1.0)
            gbp = _pst(ps, [128, 3], "mm", 2)
            te.matmul(gbp[:], ones_row32[:], gsc[:])
            gb = tail.tile([128, 3], f32, name="gb")
            sc.activation(gb[:], gbp[:], AF.Copy)
            oneminus = tail.tile([128, 1], f32, name="oneminus")
            ve.tensor_scalar(oneminus[:], gb[:, 0:1], -1.0, 1.0, ALU.mult,
                             ALU.add)
            theta = tail.tile([128, 1], f32, name="theta")
            ve.tensor_scalar_mul(theta[:], gb[:, 1:2], MEM_LR)
            eta = tail.tile([128, 1], f32, name="eta")
            ve.tensor_scalar_mul(eta[:], gb[:, 2:3], MEM_MOM)

            g1s = tail.tile([128, DIM], f32, name="g1s")
            sy.dma_start(g1s[:],
                         d["cc_g_out"][0:SL1].rearrange("(p dd) -> p dd",
                                                        p=128))
            g2s = tail.tile([64, HID], f32, name="g2s")
            sy.dma_start(g2s[:],
                         d["cc_g_out"][SL1:].rearrange("(q o) -> q o", q=64))
            mw1s = tail.tile([128, DIM], f32, name="mw1s")
            sy.dma_start(mw1s[:], d["mw1s_d"][:])
            mom1s = tail.tile([128, DIM], f32, name="mom1s")
            sy.dma_start(mom1s[:], d["mom1s_d"][:])
            mw2s = tail.tile([64, HID], f32, name="mw2s")
            sy.dma_start(mw2s[:], d["mw2s_d"][:])
            mom2s = tail.tile([64, HID], f32, name="mom2s")
            sy.dma_start(mom2s[:], d["mom2s_d"][:])

            for gsl, mws, moms, s_out, nw_out, np_ in (
                    (g1s, mw1s, mom1s, d["s1_s"], d["nw1_s"], 128),
                    (g2s, mw2s, mom2s, d["s2_s"], d["nw2_s"], 64)):
                t1 = tail.tile(list(gsl.shape), f32, name="t1", tag="t1")
                ve.tensor_scalar_mul(t1[:], gsl[:], theta[0:np_, :])
                st = tail.tile(list(gsl.shape), f32, name="st", tag="st")
                ve.scalar_tensor_tensor(st[:], moms[:], eta[0:np_, :], t1[:],
                                        ALU.mult, ALU.subtract)
                nwt = tail.tile(list(gsl.shape), f32, name="nwt", tag="nwt")
                ve.scalar_tensor_tensor(nwt[:], mws[:], oneminus[0:np_, :],
                                        st[:], ALU.mult, ALU.add)
                sy.dma_start(s_out[:], st[:])
                sy.dma_start(nw_out[:], nwt[:])


# ======================= host side =======================

def _gT(w, dt=np.float16):
    """(out, in=512) torch-Linear weight -> grouped lhsT [128, 4, out]."""
    return np.ascontiguousarray(
        w.T.reshape(GD, 128, -1).transpose(1, 0, 2)).astype(dt)


def _g_natural(w, groups, dt=np.float16):
    """(rows, cols) -> grouped [128, groups, cols] (rows on partitions)."""
    return np.ascontiguousarray(
        w.reshape(groups, 128, -1).transpose(1, 0, 2)).astype(dt)


def _prep(inputs):
    x = np.asarray(inputs["x"], np.float32)
    shared = {}
    for t, (a, b_) in {"k": ("w_k1", "w_k2"), "v": ("w_v1", "w_v2"),
                       "q": ("w_q1", "w_q2")}.items():
        shared[f"w1T_{t}"] = _gT(np.asarray(inputs[a], np.float32))
        shared[f"w2T_{t}"] = _gT(np.asarray(inputs[b_], np.float32))
    mem_w1 = np.asarray(inputs["mem_w1"], np.float32)
    mem_w2 = np.asarray(inputs["mem_w2"], np.float32)
    shared["mw1T"] = _gT(mem_w1)                       # [128,4,1024]
    shared["mw2T"] = _g_natural(mem_w2.T, GO)          # [128,8,512]
    shared["mw2n"] = _g_natural(mem_w2, GD)            # [128,4,1024]
    shared["woutT"] = _gT(np.asarray(inputs["w_out"], np.float32))
    gw = np.stack([_gT(np.asarray(inputs[f"gate_{t}_w"], np.float32),
                       np.float32)
                   for t in ("d", "lr", "m")], axis=2)  # [128,4,3,512]
    shared["gwT"] = np.ascontiguousarray(gw)
    shared["ident"] = np.eye(128, dtype=np.float16)
    shared["ones"] = np.ones((128, 129), np.float16)
    cw = np.stack([np.asarray(inputs[f"conv_{t}_w"], np.float32)[:, 0, :]
                   for t in ("k", "v", "q")], axis=1)   # (512, 3, 4)
    shared["convw"] = np.ascontiguousarray(
        cw.reshape(GD, 128, 3, K).transpose(1, 0, 2, 3))
    cb = np.stack([np.asarray(inputs[f"conv_{t}_b"], np.float32)
                   for t in ("k", "v", "q")], axis=1)   # (512, 3)
    shared["convb"] = np.ascontiguousarray(
        cb.reshape(GD, 128, 3).transpose(1, 0, 2))
    gb = np.stack([np.asarray(inputs[f"gate_{t}_b"], np.float32)
                   for t in ("d", "lr", "m")], axis=1)  # (512, 3)
    shared["gateb"] = np.ascontiguousarray(
        gb.reshape(GD, 128, 3).transpose(1, 0, 2))
    mom1 = np.asarray(inputs["mom1"], np.float32)
    mom2 = np.asarray(inputs["mom2"], np.float32)

    in_maps = []
    for c in range(N_CORES):
        b = c // 2
        half = c % 2
        seq = x[b]
        if half == 0:
            seg = np.concatenate(
                [np.zeros((PAD, DIM), np.float32), seq[0:T]], axis=0)
        else:
            seg = seq[T - PAD:2 * T]
        xTg = np.ascontiguousarray(
            seg.T.reshape(GD, 128, T + PAD).transpose(1, 0, 2)
        ).astype(np.float16)
        sel = np.zeros((128, B), np.float32)
        sel[:, b] = 1.0
        m = dict(shared)
        m["xT"] = xTg
        m["sel"] = sel
        m["mw1s"] = np.ascontiguousarray(mem_w1[c * 128:(c + 1) * 128])
        m["mom1s"] = np.ascontiguousarray(mom1[c * 128:(c + 1) * 128])
        m["mw2s"] = np.ascontiguousarray(mem_w2[c * 64:(c + 1) * 64])
        m["mom2s"] = np.ascontiguousarray(mom2[c * 64:(c + 1) * 64])
        in_maps.append(m)
    return in_maps


def _unshard(results):
    outs = []
    for c in range(N_CORES):
        a = results[c]["outT"]          # [128, 4, 4096]
        outs.append(np.moveaxis(a, 1, 0).reshape(DIM, T).T)
    output = np.concatenate(outs, axis=0).reshape(B, S, DIM)
    nw1 = np.concatenate([results[c]["nw1_s"] for c in range(N_CORES)], axis=0)
    s1 = np.concatenate([results[c]["s1_s"] for c in range(N_CORES)], axis=0)
    nw2 = np.concatenate([results[c]["nw2_s"] for c in range(N_CORES)], axis=0)
    s2 = np.concatenate([results[c]["s2_s"] for c in range(N_CORES)], axis=0)
    return (output, nw1, nw2, s1, s2)


def run(in_maps, trace=False):
    if "nc" not in _CACHE:
        _CACHE["nc"] = _build()
    return run_bass_kernel_spmd(_CACHE["nc"], in_maps,
                                core_ids=list(range(N_CORES)), trace=trace)


def kernel(**inputs):
    res = run(_prep(inputs))
    return _unshard(res.results)


if __name__ == "__main__":
    import reference as R
    inp = {k: np.asarray(v) for k, v in R.setup_inputs().items()}
    got = kernel(**inp)
    print([g.shape for g in got])
